# revision 9
# baseline (speedup 1.0000x reference)
"""Gemma3 sliding-window attention on 8 trn2 NeuronCores.

Sharding: tensor-parallel over the 8 query heads (1 head per core; each
core recomputes its KV head's k/v projection — no collectives). The host
pre-transposes/pre-tiles inputs into bf16, each core produces its head's
o_proj partial [S, HID], and the host sums the 8 partials in f32.

Device kernel (identical program on all cores, different weight data):
  phase 1 (per 128-token tile): q/k/v projections on PE, RMS-norm via
  ACT-Square+accum and a DVE fast-inverse-sqrt (avoids the ACT Sqrt
  table set so the whole kernel stays in `exp_and_others`), RoPE with
  host-folded (1+w)*cos/sin*scale tables, PE transposes of q,k into
  [d, tok] layout.
  phase 2 (per 256-token query pair): S^T = K Q^T on PE so the softmax
  needs no per-block transposes; exp(50*tanh(S^T/50)) with no
  max-subtraction (softcap bounds scores), 0/1 band masks, key-axis row
  sums via ones-matmul, attn@V and o_proj on PE, normalization deferred
  to a per-partition scale on the final psum drain.
"""

import numpy as np
import ml_dtypes

B, S, HID = 1, 2048, 2560
H, KV, D = 8, 4, 256
SCALE = 256 ** -0.5
EPS = 1e-6
P = 128
HD = D // 2          # 128, rotate_half split
NT = S // P          # 16 token tiles
KT = HID // P        # 20 contraction tiles
NPAIR = NT // 2      # 8 query-block pairs
BF16 = ml_dtypes.bfloat16

_CACHE: dict = {}


def _split_multiwait(nc):
    """walrus in this container accepts at most ONE sync wait per
    instruction; hoist extras onto wait-only EventSemaphore instructions
    inserted just before, on the same engine (same program-order
    semantics: waits are >= conditions on monotonic semaphores)."""
    import concourse.mybir as mybir

    n_new = 0
    for fn in nc.m.functions:
        for bb in fn.blocks:
            il = bb.instructions
            out = []
            for ins in il:
                si = ins.sync_info
                if si is not None and si.on_wait and len(si.on_wait) > 1:
                    waits = list(si.on_wait)
                    for w in waits[:-1]:
                        nop = mybir.InstEventSemaphore(
                            name=f"{ins.name}-hw{n_new}", ins=[], outs=[])
                        n_new += 1
                        nop.engine = ins.engine
                        nop.sync_info = mybir.SyncInfo(on_wait=[w], on_update=[])
                        nc.register_instruction(nop, overwrite=True)
                        out.append(nop)
                    ins.sync_info = mybir.SyncInfo(
                        on_wait=[waits[-1]], on_update=list(si.on_update))
                out.append(ins)
            il[:] = out


def _patch_tile_drain():
    """walrus in this container rejects multi-wait instructions; split the
    TileContext exit-drain waits into single wait_ge ops and run a
    whole-module multi-wait split pass at the very end of scheduling."""
    import concourse.mybir as mybir
    import concourse.tile as tile

    if getattr(tile.TileContext, "_drain_patched", False):
        return

    def _patched(self, tick_clock, wait_clock):
        from concourse.tile import ScopedClock

        tmp = mybir.InstNoOp(name="tmp-waits", ins=[], outs=[])
        tmp.engine = mybir.EngineType.SP
        wait_clock.add_sem_waits(tmp, ScopedClock({None: tick_clock.global_clock}))
        by_num = {h.num: h for h in self.sems.allocated().values()}
        for w in (tmp.sync_info.on_wait if tmp.sync_info else []):
            self.nc.sync.wait_ge(by_num[w.id], w.wait_value)
        self.nc.sync.drain()
        self.nc.all_engine_barrier()
        popped = self.nc._tile_sem_poison_stack.pop()
        assert popped is self._sem_poison
        self.nc.clear_and_free_semaphores(list(self.sems.allocated().values()))
        self.nc.all_engine_barrier()
        _split_multiwait(self.nc)

    tile.TileContext._drain_and_barrier = _patched
    tile.TileContext._drain_patched = True


def _build_nc():
    import concourse.bass as bass
    import concourse.mybir as mybir
    import concourse.tile as tile
    from concourse.masks import make_identity

    _patch_tile_drain()
    dt = mybir.dt
    ALU = mybir.AluOpType
    ACTF = mybir.ActivationFunctionType

    nc = bass.Bass("TRN2", target_bir_lowering=False, debug=False)

    hT = nc.dram_tensor("hT", [NT, HID, P], dt.bfloat16, kind="ExternalInput").ap()
    wq = nc.dram_tensor("wq", [HID, D], dt.bfloat16, kind="ExternalInput").ap()
    wk = nc.dram_tensor("wk", [HID, D], dt.bfloat16, kind="ExternalInput").ap()
    wv = nc.dram_tensor("wv", [HID, D], dt.bfloat16, kind="ExternalInput").ap()
    wo = nc.dram_tensor("wo", [D, HID], dt.bfloat16, kind="ExternalInput").ap()
    cq = nc.dram_tensor("cq", [S, D], dt.float32, kind="ExternalInput").ap()
    sq = nc.dram_tensor("sq", [S, D], dt.float32, kind="ExternalInput").ap()
    ck = nc.dram_tensor("ck", [S, D], dt.float32, kind="ExternalInput").ap()
    sk = nc.dram_tensor("sk", [S, D], dt.float32, kind="ExternalInput").ap()
    msk = nc.dram_tensor("msk", [4, P, 2 * P], dt.bfloat16, kind="ExternalInput").ap()
    out = nc.dram_tensor("out", [S, HID], dt.bfloat16, kind="ExternalOutput").ap()

    from contextlib import ExitStack

    with tile.TileContext(nc) as tc, ExitStack() as ctx:
        consts = ctx.enter_context(tc.tile_pool(name="consts", bufs=1))
        seq = ctx.enter_context(tc.tile_pool(name="seq", bufs=1))
        hpool = ctx.enter_context(tc.tile_pool(name="hch", bufs=2))
        tpool = ctx.enter_context(tc.tile_pool(name="tabs", bufs=2))
        work = ctx.enter_context(tc.tile_pool(name="work", bufs=3))
        small = ctx.enter_context(tc.tile_pool(name="small", bufs=3))
        ptp = ctx.enter_context(tc.tile_pool(name="ptp", bufs=2))
        outp = ctx.enter_context(tc.tile_pool(name="outp", bufs=3))
        # PSUM budget is 8 banks; every open accumulation group needs its
        # own bank (start= marks the whole 2KB zero-region pending).
        ph1_ps = ctx.enter_context(tc.tile_pool(name="ph1_ps", bufs=2, space="PSUM"))
        st_ps = ctx.enter_context(tc.tile_pool(name="st_ps", bufs=2, space="PSUM"))
        att_ps = ctx.enter_context(tc.tile_pool(name="att_ps", bufs=4, space="PSUM"))

        # ---- constants ----
        wq_sb = consts.tile([P, KT, D], dt.bfloat16, tag="wq")
        nc.sync.dma_start(out=wq_sb, in_=wq.rearrange("(t p) n -> p t n", p=P))
        wk_sb = consts.tile([P, KT, D], dt.bfloat16, tag="wk")
        nc.sync.dma_start(out=wk_sb, in_=wk.rearrange("(t p) n -> p t n", p=P))
        wv_sb = consts.tile([P, KT, D], dt.bfloat16, tag="wv")
        nc.sync.dma_start(out=wv_sb, in_=wv.rearrange("(t p) n -> p t n", p=P))
        wo_sb = consts.tile([P, 2, HID], dt.bfloat16, tag="wo")
        nc.sync.dma_start(out=wo_sb, in_=wo.rearrange("(g p) c -> p g c", p=P))
        msk_sb = consts.tile([P, 4, 2 * P], dt.bfloat16, tag="msk")
        for i in range(4):
            nc.sync.dma_start(out=msk_sb[:, i, :], in_=msk[i])
        ident = consts.tile([P, P], dt.bfloat16, tag="ident")
        make_identity(nc, ident)
        ones_col = consts.tile([P, 1], dt.bfloat16, tag="ones")
        nc.gpsimd.memset(ones_col, 1.0)

        # ---- persistent per-sequence pieces (fine-grained deps) ----
        qT_pieces = [seq.tile([P, 2, 2 * P], dt.bfloat16, tag=f"qT{i}", name=f"qT{i}")
                     for i in range(NPAIR)]
        kT_pieces = [seq.tile([P, 2, 2 * P], dt.bfloat16, tag=f"kT{i}", name=f"kT{i}")
                     for i in range(NPAIR)]
        v_pieces = [seq.tile([P, D], dt.bfloat16, tag=f"v{i}", name=f"v{i}")
                    for i in range(NT)]

        def rsqrt16(ssq):
            """[P, 2] f32 sums-of-squares -> 16/sqrt(x + 256*EPS), via
            fast-inverse-sqrt bit trick + 2 Newton iterations on DVE."""
            ms = small.tile([P, 2], dt.float32, tag="ms")
            nc.vector.tensor_scalar(ms, ssq, 256.0 * EPS, None, ALU.add)
            y = small.tile([P, 2], dt.float32, tag="y")
            yi = y.bitcast(dt.int32)
            nc.vector.tensor_scalar(yi, ms.bitcast(dt.int32), 1, None,
                                    ALU.logical_shift_right)
            nc.vector.tensor_scalar(yi, yi, -1, 0x5F3759DF, ALU.mult, ALU.add)
            t1 = small.tile([P, 2], dt.float32, tag="t1")
            for last in (False, True):
                nc.vector.tensor_mul(t1, y, y)
                nc.vector.tensor_mul(t1, t1, ms)
                if last:
                    nc.vector.tensor_scalar(t1, t1, -8.0, 24.0, ALU.mult, ALU.add)
                else:
                    nc.vector.tensor_scalar(t1, t1, -0.5, 1.5, ALU.mult, ALU.add)
                nc.vector.tensor_mul(y, y, t1)
            return y

        def rope(px, r, ctab, stab, dst):
            """dst (bf16) = (px*r)*ctab + shuffle(px*r)*stab, all on DVE."""
            a = work.tile([P, D], dt.float32, tag="ra")
            b = work.tile([P, D], dt.float32, tag="rb")
            nc.vector.scalar_tensor_tensor(a, px, r, ctab, ALU.mult, ALU.mult)
            nc.vector.scalar_tensor_tensor(b[:, 0:HD], px[:, HD:D], r,
                                           stab[:, 0:HD], ALU.mult, ALU.mult)
            nc.vector.scalar_tensor_tensor(b[:, HD:D], px[:, 0:HD], r,
                                           stab[:, HD:D], ALU.mult, ALU.mult)
            nc.vector.tensor_add(dst, a, b)

        def proj_tile(m):
            hch = hpool.tile([P, KT, P], dt.bfloat16, tag="hch")
            nc.sync.dma_start(out=hch, in_=hT[m].rearrange("(t p) n -> p t n", p=P))
            cqt = tpool.tile([P, D], dt.float32, tag="cq")
            nc.sync.dma_start(out=cqt, in_=cq[m * P:(m + 1) * P, :])
            sqt = tpool.tile([P, D], dt.float32, tag="sq")
            nc.sync.dma_start(out=sqt, in_=sq[m * P:(m + 1) * P, :])
            ckt = tpool.tile([P, D], dt.float32, tag="ck")
            nc.sync.dma_start(out=ckt, in_=ck[m * P:(m + 1) * P, :])
            skt = tpool.tile([P, D], dt.float32, tag="sk")
            nc.sync.dma_start(out=skt, in_=sk[m * P:(m + 1) * P, :])

            ssq = small.tile([P, 2], dt.float32, tag="ssq")
            pq = ph1_ps.tile([P, D], dt.float32, tag="ph1")
            for t in range(KT):
                nc.tensor.matmul(pq, lhsT=hch[:, t, :], rhs=wq_sb[:, t, :],
                                 start=(t == 0), stop=(t == KT - 1))
            scr = work.tile([P, D], dt.float32, tag="scr")
            nc.scalar.activation(scr, pq, ACTF.Square, accum_out=ssq[:, 0:1])

            pk = ph1_ps.tile([P, D], dt.float32, tag="ph1")
            for t in range(KT):
                nc.tensor.matmul(pk, lhsT=hch[:, t, :], rhs=wk_sb[:, t, :],
                                 start=(t == 0), stop=(t == KT - 1))
            scr2 = work.tile([P, D], dt.float32, tag="scr")
            nc.scalar.activation(scr2, pk, ACTF.Square, accum_out=ssq[:, 1:2])

            r16 = rsqrt16(ssq)

            qf = work.tile([P, D], dt.bfloat16, tag="qf")
            rope(pq, r16[:, 0:1], cqt, sqt, qf)   # releases pq
            pv = ph1_ps.tile([P, D], dt.float32, tag="ph1")
            for t in range(KT):
                nc.tensor.matmul(pv, lhsT=hch[:, t, :], rhs=wv_sb[:, t, :],
                                 start=(t == 0), stop=(t == KT - 1))
            kf = work.tile([P, D], dt.bfloat16, tag="kf")
            rope(pk, r16[:, 1:2], ckt, skt, kf)   # releases pk
            nc.scalar.copy(v_pieces[m], pv)

            pi, half = m // 2, (m % 2) * P
            for dh in range(2):
                tp = ph1_ps.tile([P, P], dt.bfloat16, tag="ph1", name="tp")
                nc.tensor.transpose(tp, qf[:, dh * HD:(dh + 1) * HD], ident)
                nc.scalar.copy(qT_pieces[pi][:, dh, half:half + P], tp)
                tp2 = ph1_ps.tile([P, P], dt.bfloat16, tag="ph1", name="tp2")
                nc.tensor.transpose(tp2, kf[:, dh * HD:(dh + 1) * HD], ident)
                nc.scalar.copy(kT_pieces[pi][:, dh, half:half + P], tp2)

        def attn_pair(pp):
            b = 2 * pp                      # left query block
            kk_lo = max(0, b - 4)
            kks = list(range(kk_lo, b + 2))
            n_kk = len(kks)
            qT = qT_pieces[pp]

            pt = ptp.tile([P, 6, 2 * P], dt.bfloat16, tag="pt")
            oTs = [att_ps.tile([P, 2 * P], dt.float32, tag="att", name=f"oT{dh}")
                   for dh in range(2)]
            sums = [att_ps.tile([P, 1], dt.float32, tag="att", name=f"sums{c}")
                    for c in range(2)]

            for j, kk in enumerate(kks):
                st = st_ps.tile([P, 2 * P], dt.float32, tag="st")
                for dh in range(2):
                    nc.tensor.matmul(
                        st,
                        lhsT=kT_pieces[kk // 2][:, dh, (kk % 2) * P:(kk % 2) * P + P],
                        rhs=qT[:, dh, :],
                        start=(dh == 0), stop=(dh == 1))
                th = work.tile([P, 2 * P], dt.float32, tag="th")
                nc.scalar.activation(th, st, ACTF.Tanh, scale=0.02)
                nc.scalar.activation(pt[:, j, :], th, ACTF.Exp, scale=50.0)
                rel = b - kk
                mi = {0: 1, -1: 0, 4: 2, 3: 3}.get(rel)
                if mi is not None:
                    nc.vector.tensor_mul(pt[:, j, :], pt[:, j, :], msk_sb[:, mi, :])
                for col in range(2):
                    nc.tensor.matmul(sums[col],
                                     lhsT=pt[:, j, col * P:(col + 1) * P],
                                     rhs=ones_col,
                                     start=(j == 0), stop=(j == n_kk - 1))
                for dh in range(2):
                    nc.tensor.matmul(oTs[dh],
                                     lhsT=v_pieces[kk][:, dh * P:(dh + 1) * P],
                                     rhs=pt[:, j, :],
                                     start=(j == 0), stop=(j == n_kk - 1))

            recip = small.tile([P, 2], dt.float32, tag="recip")
            nc.vector.reciprocal(recip[:, 0:1], sums[0])
            nc.vector.reciprocal(recip[:, 1:2], sums[1])
            oT_sb = work.tile([P, 2, 2 * P], dt.bfloat16, tag="oTsb")
            nc.scalar.copy(oT_sb[:, 0, :], oTs[0])
            nc.scalar.copy(oT_sb[:, 1, :], oTs[1])

            for blk in range(2):
                osb = outp.tile([P, HID], dt.bfloat16, tag="osb")
                for nch in range(5):
                    fin = att_ps.tile([P, 512], dt.float32, tag="att", name="fin")
                    for dh in range(2):
                        nc.tensor.matmul(
                            fin,
                            lhsT=oT_sb[:, dh, blk * P:(blk + 1) * P],
                            rhs=wo_sb[:, dh, nch * 512:(nch + 1) * 512],
                            start=(dh == 0), stop=(dh == 1))
                    nc.scalar.activation(osb[:, nch * 512:(nch + 1) * 512], fin,
                                         ACTF.Copy, scale=recip[:, blk:blk + 1])
                nc.sync.dma_start(out=out[(b + blk) * P:(b + blk + 1) * P, :],
                                  in_=osb)

        for m in range(NT):
            proj_tile(m)
            if m % 2 == 1:
                attn_pair(m // 2)

    return nc


def _host_prep(hidden_states, position_ids, cos_table, sin_table,
               Wq, Wk, Wv, Wo, q_norm_w, k_norm_w):
    f32 = np.float32
    hidden = np.asarray(hidden_states, f32).reshape(S, HID)
    pos = np.asarray(position_ids).reshape(B, S)[0].astype(np.int64)
    cos_g = np.asarray(cos_table, f32)[pos]          # [S, D]
    sin_g = np.asarray(sin_table, f32)[pos]
    qw = 1.0 + np.asarray(q_norm_w, f32)
    kw = 1.0 + np.asarray(k_norm_w, f32)

    sc = f32(SCALE)
    cq = (cos_g * qw * sc).astype(f32)
    sq = np.concatenate([-sin_g[:, :HD] * qw[HD:] * sc,
                         sin_g[:, HD:] * qw[:HD] * sc], axis=1).astype(f32)
    ck = (cos_g * kw).astype(f32)
    sk = np.concatenate([-sin_g[:, :HD] * kw[HD:],
                         sin_g[:, HD:] * kw[:HD]], axis=1).astype(f32)

    hT_t = np.ascontiguousarray(
        hidden.T.reshape(HID, NT, P).transpose(1, 0, 2)).astype(BF16)

    jj = np.arange(P)[:, None]
    qi = np.arange(P)[None, :]
    lt = (jj <= qi).astype(f32)
    ut = (jj > qi).astype(f32)
    z = np.zeros((P, P), f32)
    o = np.ones((P, P), f32)
    masks = np.stack([
        np.concatenate([z, lt], axis=1),   # 0: kk == b+1 (rel -1)
        np.concatenate([lt, o], axis=1),   # 1: rel 0
        np.concatenate([ut, z], axis=1),   # 2: rel 4 (left ut, right dead)
        np.concatenate([o, ut], axis=1),   # 3: rel 3 (left full, right ut)
    ]).astype(BF16)

    Wq_ = np.asarray(Wq, f32)
    Wk_ = np.asarray(Wk, f32)
    Wv_ = np.asarray(Wv, f32)
    Wo_ = np.asarray(Wo, f32)

    shared = dict(hT=hT_t, cq=cq, sq=sq, ck=ck, sk=sk, msk=masks)
    in_maps = []
    for h in range(H):
        g = h // (H // KV)
        in_maps.append(dict(
            shared,
            wq=np.ascontiguousarray(Wq_[h * D:(h + 1) * D, :].T).astype(BF16),
            wk=np.ascontiguousarray(Wk_[g * D:(g + 1) * D, :].T).astype(BF16),
            wv=np.ascontiguousarray(Wv_[g * D:(g + 1) * D, :].T).astype(BF16),
            wo=np.ascontiguousarray(Wo_[:, h * D:(h + 1) * D].T).astype(BF16),
        ))
    return in_maps


def get_nc():
    if "nc" not in _CACHE:
        _CACHE["nc"] = _build_nc()
    return _CACHE["nc"]


def kernel(hidden_states, position_ids, cos_table, sin_table,
           Wq, Wk, Wv, Wo, q_norm_w, k_norm_w):
    from concourse.bass_utils import run_bass_kernel_spmd

    nc = get_nc()
    in_maps = _host_prep(hidden_states, position_ids, cos_table, sin_table,
                         Wq, Wk, Wv, Wo, q_norm_w, k_norm_w)
    res = run_bass_kernel_spmd(nc, in_maps, list(range(H)))
    acc = np.zeros((S, HID), np.float32)
    for h in range(H):
        acc += res.results[h]["out"].astype(np.float32)
    return acc.reshape(B, S, HID)


# revision 14
# speedup vs baseline: 1.1391x; 1.1391x over previous
"""Gemma3 sliding-window attention on 8 trn2 NeuronCores.

Sharding: tensor-parallel over the 8 query heads (1 head per core; each
core recomputes its KV head's k/v projection — no collectives). The host
pre-transposes/pre-tiles inputs into bf16; each core returns its head's
UNNORMALIZED o_proj partial [S, HID] plus per-token softmax sums; the
host applies the division and sums the 8 partials in f32.

Device kernel (identical program on all cores, different weight data):
  phase 1 (per 128-token tile): fused q|k projection (N=512) + v
  projection on PE from a resident hidden^T, RMS-norm via
  ACT-Square+accum and a DVE fast-inverse-sqrt (keeps the whole kernel
  in the `exp_and_others` ACT table set), RoPE with host-folded
  (1+w)*cos/sin*scale tables, PE transposes of q,k into [d, tok].
  phase 2 (per 256-token query pair): S^T = K Q^T on PE so the softmax
  needs no per-block transposes; exp(50*tanh(S^T/50)) with no
  max-subtraction (softcap bounds scores), 0/1 band masks, key-axis
  sums via ones-matmul, attn@V and o_proj on PE, and the final psum is
  DMA'd straight to DRAM (normalization deferred to the host).
"""

import numpy as np
import ml_dtypes

B, S, HID = 1, 2048, 2560
H, KV, D = 8, 4, 256
SCALE = 256 ** -0.5
EPS = 1e-6
P = 128
HD = D // 2          # 128, rotate_half split
NT = S // P          # 16 token tiles
KT = HID // P        # 20 contraction tiles
NPAIR = NT // 2      # 8 query-block pairs
BF16 = ml_dtypes.bfloat16

_CACHE: dict = {}


def _split_multiwait(nc):
    """walrus in this container accepts at most ONE sync wait per
    instruction; hoist extras onto wait-only EventSemaphore instructions
    inserted just before, on the same engine (same program-order
    semantics: waits are >= conditions on monotonic semaphores)."""
    import concourse.mybir as mybir

    n_new = 0
    for fn in nc.m.functions:
        for bb in fn.blocks:
            il = bb.instructions
            out = []
            for ins in il:
                si = ins.sync_info
                if si is not None and si.on_wait and len(si.on_wait) > 1:
                    waits = list(si.on_wait)
                    for w in waits[:-1]:
                        nop = mybir.InstEventSemaphore(
                            name=f"{ins.name}-hw{n_new}", ins=[], outs=[])
                        n_new += 1
                        nop.engine = ins.engine
                        nop.sync_info = mybir.SyncInfo(on_wait=[w], on_update=[])
                        nc.register_instruction(nop, overwrite=True)
                        out.append(nop)
                    ins.sync_info = mybir.SyncInfo(
                        on_wait=[waits[-1]], on_update=list(si.on_update))
                out.append(ins)
            il[:] = out


def _patch_tile_drain():
    """walrus in this container rejects multi-wait instructions; split the
    TileContext exit-drain waits into single wait_ge ops and run a
    whole-module multi-wait split pass at the very end of scheduling."""
    import concourse.mybir as mybir
    import concourse.tile as tile

    if getattr(tile.TileContext, "_drain_patched", False):
        return

    def _patched(self, tick_clock, wait_clock):
        from concourse.tile import ScopedClock

        tmp = mybir.InstNoOp(name="tmp-waits", ins=[], outs=[])
        tmp.engine = mybir.EngineType.SP
        wait_clock.add_sem_waits(tmp, ScopedClock({None: tick_clock.global_clock}))
        by_num = {h.num: h for h in self.sems.allocated().values()}
        for w in (tmp.sync_info.on_wait if tmp.sync_info else []):
            self.nc.sync.wait_ge(by_num[w.id], w.wait_value)
        self.nc.sync.drain()
        self.nc.all_engine_barrier()
        popped = self.nc._tile_sem_poison_stack.pop()
        assert popped is self._sem_poison
        self.nc.clear_and_free_semaphores(list(self.sems.allocated().values()))
        self.nc.all_engine_barrier()
        _split_multiwait(self.nc)

    tile.TileContext._drain_and_barrier = _patched
    tile.TileContext._drain_patched = True


def _build_nc():
    import concourse.bass as bass
    import concourse.mybir as mybir
    import concourse.tile as tile
    from concourse.masks import make_identity

    _patch_tile_drain()
    dt = mybir.dt
    ALU = mybir.AluOpType
    ACTF = mybir.ActivationFunctionType

    nc = bass.Bass("TRN2", target_bir_lowering=False, debug=False)

    hT = nc.dram_tensor("hT", [HID, S], dt.bfloat16, kind="ExternalInput").ap()
    wqk = nc.dram_tensor("wqk", [HID, 2 * D], dt.bfloat16, kind="ExternalInput").ap()
    wv = nc.dram_tensor("wv", [HID, D], dt.bfloat16, kind="ExternalInput").ap()
    wo = nc.dram_tensor("wo", [D, HID], dt.bfloat16, kind="ExternalInput").ap()
    cq = nc.dram_tensor("cq", [S, D], dt.float32, kind="ExternalInput").ap()
    sq = nc.dram_tensor("sq", [S, D], dt.float32, kind="ExternalInput").ap()
    ck = nc.dram_tensor("ck", [S, D], dt.float32, kind="ExternalInput").ap()
    sk = nc.dram_tensor("sk", [S, D], dt.float32, kind="ExternalInput").ap()
    msk = nc.dram_tensor("msk", [4, P, 2 * P], dt.bfloat16, kind="ExternalInput").ap()
    out = nc.dram_tensor("out", [S, HID], dt.bfloat16, kind="ExternalOutput").ap()
    sums_d = nc.dram_tensor("sums", [S], dt.float32, kind="ExternalOutput").ap()

    from contextlib import ExitStack

    with tile.TileContext(nc) as tc, ExitStack() as ctx:
        consts = ctx.enter_context(tc.tile_pool(name="consts", bufs=1))
        seq = ctx.enter_context(tc.tile_pool(name="seq", bufs=1))
        tpool = ctx.enter_context(tc.tile_pool(name="tabs", bufs=2))
        work = ctx.enter_context(tc.tile_pool(name="work", bufs=3))
        small = ctx.enter_context(tc.tile_pool(name="small", bufs=3))
        ptp = ctx.enter_context(tc.tile_pool(name="ptp", bufs=2))
        outp = ctx.enter_context(tc.tile_pool(name="outp", bufs=3))
        # PSUM budget is 8 banks; every open accumulation group needs its
        # own bank (start= marks the whole 2KB zero-region pending).
        ph1_ps = ctx.enter_context(tc.tile_pool(name="ph1_ps", bufs=2, space="PSUM"))
        st_ps = ctx.enter_context(tc.tile_pool(name="st_ps", bufs=2, space="PSUM"))
        att_ps = ctx.enter_context(tc.tile_pool(name="att_ps", bufs=4, space="PSUM"))

        # ---- constants / resident inputs ----
        # hidden^T resident in SBUF, 4 token-quarter DMAs (1KB segments)
        hq = []
        for i in range(4):
            hqt = consts.tile([P, KT, S // 4], dt.bfloat16, tag=f"hq{i}",
                              name=f"hq{i}")
            nc.sync.dma_start(
                out=hqt,
                in_=hT[:, i * (S // 4):(i + 1) * (S // 4)]
                .rearrange("(t p) n -> p t n", p=P))
            hq.append(hqt)

        # q|k fused weights, 4 contraction-groups so early matmuls start
        # as soon as the first group lands
        wqk_sb = []
        for g in range(4):
            wt = consts.tile([P, 5, 2 * D], dt.bfloat16, tag=f"wqk{g}",
                             name=f"wqk{g}")
            nc.sync.dma_start(
                out=wt,
                in_=wqk[g * 5 * P:(g + 1) * 5 * P, :]
                .rearrange("(t p) n -> p t n", p=P))
            wqk_sb.append(wt)
        wv_sb = []
        for g in range(4):
            wt = consts.tile([P, 5, D], dt.bfloat16, tag=f"wv{g}", name=f"wv{g}")
            nc.sync.dma_start(
                out=wt,
                in_=wv[g * 5 * P:(g + 1) * 5 * P, :]
                .rearrange("(t p) n -> p t n", p=P))
            wv_sb.append(wt)
        wo_sb = consts.tile([P, 2, HID], dt.bfloat16, tag="wo")
        nc.sync.dma_start(out=wo_sb, in_=wo.rearrange("(g p) c -> p g c", p=P))
        msk_sb = consts.tile([P, 4, 2 * P], dt.bfloat16, tag="msk")
        for i in range(4):
            nc.sync.dma_start(out=msk_sb[:, i, :], in_=msk[i])
        ident = consts.tile([P, P], dt.bfloat16, tag="ident")
        make_identity(nc, ident)
        ones_col = consts.tile([P, 1], dt.bfloat16, tag="ones")
        nc.gpsimd.memset(ones_col, 1.0)

        # ---- persistent per-sequence pieces (fine-grained deps) ----
        qT_pieces = [seq.tile([P, 2, 2 * P], dt.bfloat16, tag=f"qT{i}", name=f"qT{i}")
                     for i in range(NPAIR)]
        kT_pieces = [seq.tile([P, 2, 2 * P], dt.bfloat16, tag=f"kT{i}", name=f"kT{i}")
                     for i in range(NPAIR)]
        v_pieces = [seq.tile([P, D], dt.bfloat16, tag=f"v{i}", name=f"v{i}")
                    for i in range(NT)]

        def rsqrt16(ssq):
            """[P, 2] f32 sums-of-squares -> 16/sqrt(x + 256*EPS), via
            fast-inverse-sqrt bit trick + 2 Newton iterations on DVE."""
            ms = small.tile([P, 2], dt.float32, tag="ms")
            nc.vector.tensor_scalar(ms, ssq, 256.0 * EPS, None, ALU.add)
            y = small.tile([P, 2], dt.float32, tag="y")
            yi = y.bitcast(dt.int32)
            nc.vector.tensor_scalar(yi, ms.bitcast(dt.int32), 1, None,
                                    ALU.logical_shift_right)
            nc.vector.tensor_scalar(yi, yi, -1, 0x5F3759DF, ALU.mult, ALU.add)
            t1 = small.tile([P, 2], dt.float32, tag="t1")
            for last in (False, True):
                nc.vector.tensor_mul(t1, y, y)
                nc.vector.tensor_mul(t1, t1, ms)
                if last:
                    nc.vector.tensor_scalar(t1, t1, -8.0, 24.0, ALU.mult, ALU.add)
                else:
                    nc.vector.tensor_scalar(t1, t1, -0.5, 1.5, ALU.mult, ALU.add)
                nc.vector.tensor_mul(y, y, t1)
            return y

        def rope(px, r, ctab, stab, dst):
            """dst (bf16) = (px*r)*ctab + shuffle(px*r)*stab, all on DVE."""
            a = work.tile([P, D], dt.float32, tag="ra")
            b = work.tile([P, D], dt.float32, tag="rb")
            nc.vector.scalar_tensor_tensor(a, px, r, ctab, ALU.mult, ALU.mult)
            nc.vector.scalar_tensor_tensor(b[:, 0:HD], px[:, HD:D], r,
                                           stab[:, 0:HD], ALU.mult, ALU.mult)
            nc.vector.scalar_tensor_tensor(b[:, HD:D], px[:, 0:HD], r,
                                           stab[:, HD:D], ALU.mult, ALU.mult)
            nc.vector.tensor_add(dst, a, b)

        def proj_tile(m):
            quarter = hq[m // 4]
            toff = (m % 4) * P
            cqt = tpool.tile([P, D], dt.float32, tag="cq")
            nc.sync.dma_start(out=cqt, in_=cq[m * P:(m + 1) * P, :])
            sqt = tpool.tile([P, D], dt.float32, tag="sq")
            nc.sync.dma_start(out=sqt, in_=sq[m * P:(m + 1) * P, :])
            ckt = tpool.tile([P, D], dt.float32, tag="ck")
            nc.sync.dma_start(out=ckt, in_=ck[m * P:(m + 1) * P, :])
            skt = tpool.tile([P, D], dt.float32, tag="sk")
            nc.sync.dma_start(out=skt, in_=sk[m * P:(m + 1) * P, :])

            ssq = small.tile([P, 2], dt.float32, tag="ssq")
            pqk = ph1_ps.tile([P, 2 * D], dt.float32, tag="ph1", name="pqk")
            for t in range(KT):
                nc.tensor.matmul(pqk, lhsT=quarter[:, t, toff:toff + P],
                                 rhs=wqk_sb[t // 5][:, t % 5, :],
                                 start=(t == 0), stop=(t == KT - 1))
            pv = ph1_ps.tile([P, D], dt.float32, tag="ph1", name="pv")
            for t in range(KT):
                nc.tensor.matmul(pv, lhsT=quarter[:, t, toff:toff + P],
                                 rhs=wv_sb[t // 5][:, t % 5, :],
                                 start=(t == 0), stop=(t == KT - 1))

            scr = work.tile([P, D], dt.float32, tag="scr")
            nc.scalar.activation(scr, pqk[:, 0:D], ACTF.Square,
                                 accum_out=ssq[:, 0:1])
            scr2 = work.tile([P, D], dt.float32, tag="scr")
            nc.scalar.activation(scr2, pqk[:, D:2 * D], ACTF.Square,
                                 accum_out=ssq[:, 1:2])

            r16 = rsqrt16(ssq)

            qf = work.tile([P, D], dt.bfloat16, tag="qf")
            rope(pqk[:, 0:D], r16[:, 0:1], cqt, sqt, qf)
            kf = work.tile([P, D], dt.bfloat16, tag="kf")
            rope(pqk[:, D:2 * D], r16[:, 1:2], ckt, skt, kf)
            nc.vector.tensor_copy(v_pieces[m], pv)

            pi, half = m // 2, (m % 2) * P
            for dh in range(2):
                tp = ph1_ps.tile([P, P], dt.bfloat16, tag="ph1", name="tp")
                nc.tensor.transpose(tp, qf[:, dh * HD:(dh + 1) * HD], ident)
                nc.vector.tensor_copy(qT_pieces[pi][:, dh, half:half + P], tp)
                tp2 = ph1_ps.tile([P, P], dt.bfloat16, tag="ph1", name="tp2")
                nc.tensor.transpose(tp2, kf[:, dh * HD:(dh + 1) * HD], ident)
                nc.vector.tensor_copy(kT_pieces[pi][:, dh, half:half + P], tp2)

        def pair_kks(pp):
            b = 2 * pp
            return b, list(range(max(0, b - 4), b + 2))

        def attn_scores(pp):
            """Produce the masked exp(softcap) probabilities P^T for pair
            pp into its pt tile."""
            b, kks = pair_kks(pp)
            qT = qT_pieces[pp]
            pt = ptp.tile([P, 6, 2 * P], dt.bfloat16, tag="pt", name=f"pt{pp % 2}")
            for j, kk in enumerate(kks):
                st = st_ps.tile([P, 2 * P], dt.float32, tag="st")
                for dh in range(2):
                    nc.tensor.matmul(
                        st,
                        lhsT=kT_pieces[kk // 2][:, dh, (kk % 2) * P:(kk % 2) * P + P],
                        rhs=qT[:, dh, :],
                        start=(dh == 0), stop=(dh == 1))
                th = work.tile([P, 2 * P], dt.float32, tag="th")
                nc.scalar.activation(th, st, ACTF.Tanh, scale=0.02)
                nc.scalar.activation(pt[:, j, :], th, ACTF.Exp, scale=50.0)
                rel = b - kk
                mi = {0: 1, -1: 0, 4: 2, 3: 3}.get(rel)
                if mi is not None:
                    nc.vector.tensor_mul(pt[:, j, :], pt[:, j, :], msk_sb[:, mi, :])
            return pt

        def attn_out(pp, pt):
            """Key-sums, attn@V, o_proj; unnormalized psum -> DRAM."""
            b, kks = pair_kks(pp)
            n_kk = len(kks)
            oTs = [att_ps.tile([P, 2 * P], dt.float32, tag="att", name=f"oT{dh}")
                   for dh in range(2)]
            sums = [att_ps.tile([P, 1], dt.float32, tag="att", name=f"sums{c}")
                    for c in range(2)]
            for j, kk in enumerate(kks):
                for col in range(2):
                    nc.tensor.matmul(sums[col],
                                     lhsT=pt[:, j, col * P:(col + 1) * P],
                                     rhs=ones_col,
                                     start=(j == 0), stop=(j == n_kk - 1))
                for dh in range(2):
                    nc.tensor.matmul(oTs[dh],
                                     lhsT=v_pieces[kk][:, dh * P:(dh + 1) * P],
                                     rhs=pt[:, j, :],
                                     start=(j == 0), stop=(j == n_kk - 1))
            sums_sb = small.tile([P, 2], dt.float32, tag="sums_sb")
            nc.vector.tensor_copy(sums_sb[:, 0:1], sums[0])
            nc.vector.tensor_copy(sums_sb[:, 1:2], sums[1])
            for col in range(2):
                nc.sync.dma_start(out=sums_d[(b + col) * P:(b + col + 1) * P],
                                  in_=sums_sb[:, col:col + 1])
            oT_sb = work.tile([P, 2, 2 * P], dt.bfloat16, tag="oTsb")
            nc.vector.tensor_copy(oT_sb[:, 0, :], oTs[0])
            nc.vector.tensor_copy(oT_sb[:, 1, :], oTs[1])

            for blk in range(2):
                osb = outp.tile([P, HID], dt.bfloat16, tag="osb")
                for nch in range(5):
                    fin = att_ps.tile([P, 512], dt.float32, tag="att", name="fin")
                    for dh in range(2):
                        nc.tensor.matmul(
                            fin,
                            lhsT=oT_sb[:, dh, blk * P:(blk + 1) * P],
                            rhs=wo_sb[:, dh, nch * 512:(nch + 1) * 512],
                            start=(dh == 0), stop=(dh == 1))
                    dst = osb[:, nch * 512:(nch + 1) * 512]
                    if nch % 2 == 0:
                        nc.vector.tensor_copy(dst, fin)
                    else:
                        nc.scalar.copy(dst, fin)
                nc.sync.dma_start(out=out[(b + blk) * P:(b + blk + 1) * P, :],
                                  in_=osb)

        # schedule: keep PE fed — next pair's scores are emitted before
        # this pair's output stage so fins never head-block the PE queue
        pts = {}
        for m in range(NT):
            proj_tile(m)
            if m % 2 == 1:
                pp = m // 2
                pts[pp] = attn_scores(pp)
                if pp >= 1:
                    attn_out(pp - 1, pts.pop(pp - 1))
        attn_out(NPAIR - 1, pts.pop(NPAIR - 1))

    return nc


def _host_prep(hidden_states, position_ids, cos_table, sin_table,
               Wq, Wk, Wv, Wo, q_norm_w, k_norm_w):
    f32 = np.float32
    hidden = np.asarray(hidden_states, f32).reshape(S, HID)
    pos = np.asarray(position_ids).reshape(B, S)[0].astype(np.int64)
    cos_g = np.asarray(cos_table, f32)[pos]          # [S, D]
    sin_g = np.asarray(sin_table, f32)[pos]
    qw = 1.0 + np.asarray(q_norm_w, f32)
    kw = 1.0 + np.asarray(k_norm_w, f32)

    sc = f32(SCALE)
    cq = (cos_g * qw * sc).astype(f32)
    sq = np.concatenate([-sin_g[:, :HD] * qw[HD:] * sc,
                         sin_g[:, HD:] * qw[:HD] * sc], axis=1).astype(f32)
    ck = (cos_g * kw).astype(f32)
    sk = np.concatenate([-sin_g[:, :HD] * kw[HD:],
                         sin_g[:, HD:] * kw[:HD]], axis=1).astype(f32)

    hT_t = np.ascontiguousarray(hidden.T).astype(BF16)   # [HID, S]

    jj = np.arange(P)[:, None]
    qi = np.arange(P)[None, :]
    lt = (jj <= qi).astype(f32)
    ut = (jj > qi).astype(f32)
    z = np.zeros((P, P), f32)
    o = np.ones((P, P), f32)
    masks = np.stack([
        np.concatenate([z, lt], axis=1),   # 0: kk == b+1 (rel -1)
        np.concatenate([lt, o], axis=1),   # 1: rel 0
        np.concatenate([ut, z], axis=1),   # 2: rel 4 (left ut, right dead)
        np.concatenate([o, ut], axis=1),   # 3: rel 3 (left full, right ut)
    ]).astype(BF16)

    Wq_ = np.asarray(Wq, f32)
    Wk_ = np.asarray(Wk, f32)
    Wv_ = np.asarray(Wv, f32)
    Wo_ = np.asarray(Wo, f32)

    shared = dict(hT=hT_t, cq=cq, sq=sq, ck=ck, sk=sk, msk=masks)
    in_maps = []
    for h in range(H):
        g = h // (H // KV)
        wq_h = Wq_[h * D:(h + 1) * D, :].T          # [HID, D]
        wk_g = Wk_[g * D:(g + 1) * D, :].T
        in_maps.append(dict(
            shared,
            wqk=np.ascontiguousarray(
                np.concatenate([wq_h, wk_g], axis=1)).astype(BF16),
            wv=np.ascontiguousarray(Wv_[g * D:(g + 1) * D, :].T).astype(BF16),
            wo=np.ascontiguousarray(Wo_[:, h * D:(h + 1) * D].T).astype(BF16),
        ))
    return in_maps


def get_nc():
    if "nc" not in _CACHE:
        _CACHE["nc"] = _build_nc()
    return _CACHE["nc"]


def kernel(hidden_states, position_ids, cos_table, sin_table,
           Wq, Wk, Wv, Wo, q_norm_w, k_norm_w):
    from concourse.bass_utils import run_bass_kernel_spmd

    nc = get_nc()
    in_maps = _host_prep(hidden_states, position_ids, cos_table, sin_table,
                         Wq, Wk, Wv, Wo, q_norm_w, k_norm_w)
    res = run_bass_kernel_spmd(nc, in_maps, list(range(H)))
    acc = np.zeros((S, HID), np.float32)
    for h in range(H):
        r = res.results[h]
        acc += r["out"].astype(np.float32) * (1.0 / r["sums"])[:, None]
    return acc.reshape(B, S, HID)


# revision 19
# speedup vs baseline: 1.3076x; 1.1480x over previous
"""Gemma3 sliding-window attention on 8 trn2 NeuronCores.

Sharding: tensor-parallel over the 8 query heads (1 head per core; each
core recomputes its KV head's k/v projection — no collectives). The host
pre-transposes/pre-tiles inputs into bf16; each core returns its head's
UNNORMALIZED o_proj partial [S, HID] plus per-token softmax sums; the
host applies the division and sums the 8 partials in f32.

Device kernel (identical program on all cores, different weight data):
  phase 1 (per 128-token tile): fused q|k projection (N=512) + v
  projection on PE from a resident hidden^T, RMS-norm via
  ACT-Square+accum and a DVE fast-inverse-sqrt (keeps the whole kernel
  in the `exp_and_others` ACT table set), RoPE with host-folded
  (1+w)*cos/sin*scale tables, PE transposes of q,k into [d, tok].
  phase 2 (per 256-token query pair): S^T = K Q^T on PE so the softmax
  needs no per-block transposes; exp(50*tanh(S^T/50)) with no
  max-subtraction (softcap bounds scores), 0/1 band masks, key-axis
  sums via ones-matmul, attn@V and o_proj on PE, and the final psum is
  DMA'd straight to DRAM (normalization deferred to the host).
"""

import numpy as np
import ml_dtypes

B, S, HID = 1, 2048, 2560
H, KV, D = 8, 4, 256
SCALE = 256 ** -0.5
EPS = 1e-6
P = 128
HD = D // 2          # 128, rotate_half split
NT = S // P          # 16 token tiles
KT = HID // P        # 20 contraction tiles
NPAIR = NT // 2      # 8 query-block pairs
BF16 = ml_dtypes.bfloat16

_CACHE: dict = {}


def _split_multiwait(nc):
    """walrus in this container accepts at most ONE sync wait per
    instruction; hoist extras onto wait-only EventSemaphore instructions
    inserted just before, on the same engine (same program-order
    semantics: waits are >= conditions on monotonic semaphores)."""
    import concourse.mybir as mybir

    n_new = 0
    for fn in nc.m.functions:
        for bb in fn.blocks:
            il = bb.instructions
            out = []
            for ins in il:
                si = ins.sync_info
                if si is not None and si.on_wait and len(si.on_wait) > 1:
                    waits = list(si.on_wait)
                    for w in waits[:-1]:
                        nop = mybir.InstEventSemaphore(
                            name=f"{ins.name}-hw{n_new}", ins=[], outs=[])
                        n_new += 1
                        nop.engine = ins.engine
                        nop.sync_info = mybir.SyncInfo(on_wait=[w], on_update=[])
                        nc.register_instruction(nop, overwrite=True)
                        out.append(nop)
                    ins.sync_info = mybir.SyncInfo(
                        on_wait=[waits[-1]], on_update=list(si.on_update))
                out.append(ins)
            il[:] = out


def _patch_tile_drain():
    """walrus in this container rejects multi-wait instructions; split the
    TileContext exit-drain waits into single wait_ge ops and run a
    whole-module multi-wait split pass at the very end of scheduling."""
    import concourse.mybir as mybir
    import concourse.tile as tile

    if getattr(tile.TileContext, "_drain_patched", False):
        return

    def _patched(self, tick_clock, wait_clock):
        from concourse.tile import ScopedClock

        tmp = mybir.InstNoOp(name="tmp-waits", ins=[], outs=[])
        tmp.engine = mybir.EngineType.SP
        wait_clock.add_sem_waits(tmp, ScopedClock({None: tick_clock.global_clock}))
        by_num = {h.num: h for h in self.sems.allocated().values()}
        for w in (tmp.sync_info.on_wait if tmp.sync_info else []):
            self.nc.sync.wait_ge(by_num[w.id], w.wait_value)
        self.nc.sync.drain()
        self.nc.all_engine_barrier()
        popped = self.nc._tile_sem_poison_stack.pop()
        assert popped is self._sem_poison
        self.nc.clear_and_free_semaphores(list(self.sems.allocated().values()))
        self.nc.all_engine_barrier()
        _split_multiwait(self.nc)

    tile.TileContext._drain_and_barrier = _patched
    tile.TileContext._drain_patched = True


def _build_nc():
    import concourse.bass as bass
    import concourse.mybir as mybir
    import concourse.tile as tile
    from concourse.masks import make_identity

    _patch_tile_drain()
    dt = mybir.dt
    ALU = mybir.AluOpType
    ACTF = mybir.ActivationFunctionType

    nc = bass.Bass("TRN2", target_bir_lowering=False, debug=False)

    hT = nc.dram_tensor("hT", [HID, S], dt.bfloat16, kind="ExternalInput").ap()
    wqk = nc.dram_tensor("wqk", [HID, 2 * D], dt.bfloat16, kind="ExternalInput").ap()
    wv = nc.dram_tensor("wv", [HID, D], dt.bfloat16, kind="ExternalInput").ap()
    wo = nc.dram_tensor("wo", [D, HID], dt.bfloat16, kind="ExternalInput").ap()
    cq = nc.dram_tensor("cq", [S, D], dt.float32, kind="ExternalInput").ap()
    sq = nc.dram_tensor("sq", [S, D], dt.float32, kind="ExternalInput").ap()
    ck = nc.dram_tensor("ck", [S, D], dt.float32, kind="ExternalInput").ap()
    sk = nc.dram_tensor("sk", [S, D], dt.float32, kind="ExternalInput").ap()
    msk = nc.dram_tensor("msk", [4, P, 2 * P], dt.bfloat16, kind="ExternalInput").ap()
    out = nc.dram_tensor("out", [S, HID], dt.bfloat16, kind="ExternalOutput").ap()
    sums_d = nc.dram_tensor("sums", [S], dt.float32, kind="ExternalOutput").ap()

    from contextlib import ExitStack

    with tile.TileContext(nc) as tc, ExitStack() as ctx:
        consts = ctx.enter_context(tc.tile_pool(name="consts", bufs=1))
        seq = ctx.enter_context(tc.tile_pool(name="seq", bufs=1))
        tpool = ctx.enter_context(tc.tile_pool(name="tabs", bufs=2))
        work = ctx.enter_context(tc.tile_pool(name="work", bufs=3))
        small = ctx.enter_context(tc.tile_pool(name="small", bufs=3))
        ptp = ctx.enter_context(tc.tile_pool(name="ptp", bufs=2))
        outp = ctx.enter_context(tc.tile_pool(name="outp", bufs=3))
        # PSUM budget is 8 banks; every open accumulation group needs its
        # own bank (start= marks the whole 2KB zero-region pending).
        ph1_ps = ctx.enter_context(tc.tile_pool(name="ph1_ps", bufs=2, space="PSUM"))
        st_ps = ctx.enter_context(tc.tile_pool(name="st_ps", bufs=2, space="PSUM"))
        att_ps = ctx.enter_context(tc.tile_pool(name="att_ps", bufs=4, space="PSUM"))

        # ---- constants / resident inputs ----
        # DMA emission is staggered: only what tile 0 needs goes first
        # (quarter 0 of hidden^T + weights); the rest is emitted inside
        # the main loop so the initial burst doesn't starve tile 0.
        hq = [consts.tile([P, KT, S // 4], dt.bfloat16, tag=f"hq{i}",
                          name=f"hq{i}") for i in range(4)]

        def load_quarter(i):
            nc.sync.dma_start(
                out=hq[i],
                in_=hT[:, i * (S // 4):(i + 1) * (S // 4)]
                .rearrange("(t p) n -> p t n", p=P))

        # q|k fused weights, 4 contraction-groups so early matmuls start
        # as soon as the first group lands
        wqk_sb = []
        for g in range(4):
            wt = consts.tile([P, 5, 2 * D], dt.bfloat16, tag=f"wqk{g}",
                             name=f"wqk{g}")
            nc.sync.dma_start(
                out=wt,
                in_=wqk[g * 5 * P:(g + 1) * 5 * P, :]
                .rearrange("(t p) n -> p t n", p=P))
            wqk_sb.append(wt)
        wv_sb = []
        for g in range(4):
            wt = consts.tile([P, 5, D], dt.bfloat16, tag=f"wv{g}", name=f"wv{g}")
            nc.sync.dma_start(
                out=wt,
                in_=wv[g * 5 * P:(g + 1) * 5 * P, :]
                .rearrange("(t p) n -> p t n", p=P))
            wv_sb.append(wt)
        load_quarter(0)
        wo_sb = consts.tile([P, 2, HID], dt.bfloat16, tag="wo")
        msk_sb = consts.tile([P, 4, 2 * P], dt.bfloat16, tag="msk")
        ident = consts.tile([P, P], dt.bfloat16, tag="ident")
        make_identity(nc, ident)
        ones_col = consts.tile([P, 1], dt.bfloat16, tag="ones")
        nc.gpsimd.memset(ones_col, 1.0)

        # ---- persistent per-sequence pieces (fine-grained deps) ----
        qT_pieces = [seq.tile([P, 2, 2 * P], dt.bfloat16, tag=f"qT{i}", name=f"qT{i}")
                     for i in range(NPAIR)]
        kT_pieces = [seq.tile([P, 2, 2 * P], dt.bfloat16, tag=f"kT{i}", name=f"kT{i}")
                     for i in range(NPAIR)]
        v_pieces = [seq.tile([P, D], dt.bfloat16, tag=f"v{i}", name=f"v{i}")
                    for i in range(NT)]

        def rsqrt16(ssq):
            """[P, 2] f32 sums-of-squares -> 16/sqrt(x + 256*EPS), via
            fast-inverse-sqrt bit trick + 2 Newton iterations on DVE.
            Two Halley-free Newton steps folded: the second step carries
            the x16 scale. One step leaves ~1.7e-3 rel err (fine next to
            bf16), so only one is used."""
            ms = small.tile([P, 2], dt.float32, tag="ms")
            nc.vector.tensor_scalar(ms, ssq, 256.0 * EPS, None, ALU.add)
            y = small.tile([P, 2], dt.float32, tag="y")
            yi = y.bitcast(dt.int32)
            nc.vector.tensor_scalar(yi, ms.bitcast(dt.int32), 1, None,
                                    ALU.logical_shift_right)
            nc.vector.tensor_scalar(yi, yi, -1, 0x5F3759DF, ALU.mult, ALU.add)
            t1 = small.tile([P, 2], dt.float32, tag="t1")
            nc.vector.tensor_mul(t1, y, y)
            nc.vector.tensor_mul(t1, t1, ms)
            nc.vector.tensor_scalar(t1, t1, -8.0, 24.0, ALU.mult, ALU.add)
            nc.vector.tensor_mul(y, y, t1)
            return y

        def rope(px, r, ctab, stab, dst):
            """dst (bf16) = (px*r)*ctab + shuffle(px*r)*stab, all on DVE."""
            a = work.tile([P, D], dt.float32, tag="ra")
            b = work.tile([P, D], dt.float32, tag="rb")
            nc.vector.scalar_tensor_tensor(a, px, r, ctab, ALU.mult, ALU.mult)
            nc.vector.scalar_tensor_tensor(b[:, 0:HD], px[:, HD:D], r,
                                           stab[:, 0:HD], ALU.mult, ALU.mult)
            nc.vector.scalar_tensor_tensor(b[:, HD:D], px[:, 0:HD], r,
                                           stab[:, HD:D], ALU.mult, ALU.mult)
            nc.vector.tensor_add(dst, a, b)

        def proj_matmuls(m):
            quarter = hq[m // 4]
            toff = (m % 4) * P
            cqt = tpool.tile([P, D], dt.float32, tag="cq")
            nc.sync.dma_start(out=cqt, in_=cq[m * P:(m + 1) * P, :])
            sqt = tpool.tile([P, D], dt.float32, tag="sq")
            nc.sync.dma_start(out=sqt, in_=sq[m * P:(m + 1) * P, :])
            ckt = tpool.tile([P, D], dt.float32, tag="ck")
            nc.sync.dma_start(out=ckt, in_=ck[m * P:(m + 1) * P, :])
            skt = tpool.tile([P, D], dt.float32, tag="sk")
            nc.sync.dma_start(out=skt, in_=sk[m * P:(m + 1) * P, :])

            pqk = ph1_ps.tile([P, 2 * D], dt.float32, tag="ph1", name="pqk")
            for t in range(KT):
                nc.tensor.matmul(pqk, lhsT=quarter[:, t, toff:toff + P],
                                 rhs=wqk_sb[t // 5][:, t % 5, :],
                                 start=(t == 0), stop=(t == KT - 1))
            pv = ph1_ps.tile([P, D], dt.float32, tag="ph1", name="pv")
            for t in range(KT):
                nc.tensor.matmul(pv, lhsT=quarter[:, t, toff:toff + P],
                                 rhs=wv_sb[t // 5][:, t % 5, :],
                                 start=(t == 0), stop=(t == KT - 1))
            return pqk, pv, cqt, sqt, ckt, skt

        def norm_rope_tp(m, pqk, pv, cqt, sqt, ckt, skt):
            ssq = small.tile([P, 2], dt.float32, tag="ssq")
            scr = work.tile([P, D], dt.float32, tag="scr")
            nc.scalar.activation(scr, pqk[:, 0:D], ACTF.Square,
                                 accum_out=ssq[:, 0:1])
            scr2 = work.tile([P, D], dt.float32, tag="scr")
            nc.scalar.activation(scr2, pqk[:, D:2 * D], ACTF.Square,
                                 accum_out=ssq[:, 1:2])

            r16 = rsqrt16(ssq)

            qf = work.tile([P, D], dt.bfloat16, tag="qf")
            rope(pqk[:, 0:D], r16[:, 0:1], cqt, sqt, qf)
            kf = work.tile([P, D], dt.bfloat16, tag="kf")
            rope(pqk[:, D:2 * D], r16[:, 1:2], ckt, skt, kf)
            nc.vector.tensor_copy(v_pieces[m], pv)

            pi, half = m // 2, (m % 2) * P
            for dh in range(2):
                tp = ph1_ps.tile([P, P], dt.bfloat16, tag="ph1", name="tp")
                nc.tensor.transpose(tp, qf[:, dh * HD:(dh + 1) * HD], ident)
                nc.vector.tensor_copy(qT_pieces[pi][:, dh, half:half + P], tp)
                tp2 = ph1_ps.tile([P, P], dt.bfloat16, tag="ph1", name="tp2")
                nc.tensor.transpose(tp2, kf[:, dh * HD:(dh + 1) * HD], ident)
                nc.vector.tensor_copy(kT_pieces[pi][:, dh, half:half + P], tp2)

        def pair_kks(pp):
            b = 2 * pp
            return b, list(range(max(0, b - 4), b + 2))

        def attn_scores(pp):
            """Produce the masked exp(softcap) probabilities P^T for pair
            pp into its pt tile."""
            b, kks = pair_kks(pp)
            qT = qT_pieces[pp]
            pt = ptp.tile([P, 6, 2 * P], dt.bfloat16, tag="pt", name=f"pt{pp % 2}")
            for j, kk in enumerate(kks):
                st = st_ps.tile([P, 2 * P], dt.float32, tag="st")
                for dh in range(2):
                    nc.tensor.matmul(
                        st,
                        lhsT=kT_pieces[kk // 2][:, dh, (kk % 2) * P:(kk % 2) * P + P],
                        rhs=qT[:, dh, :],
                        start=(dh == 0), stop=(dh == 1))
                th = work.tile([P, 2 * P], dt.float32, tag="th")
                nc.scalar.activation(th, st, ACTF.Tanh, scale=0.02)
                nc.scalar.activation(pt[:, j, :], th, ACTF.Exp, scale=50.0)
                rel = b - kk
                mi = {0: 1, -1: 0, 4: 2, 3: 3}.get(rel)
                if mi is not None:
                    nc.vector.tensor_mul(pt[:, j, :], pt[:, j, :], msk_sb[:, mi, :])
            return pt

        def attn_out(pp, pt):
            """Key-sums, attn@V, o_proj; unnormalized psum -> DRAM."""
            b, kks = pair_kks(pp)
            n_kk = len(kks)
            oTs = [att_ps.tile([P, 2 * P], dt.float32, tag="att", name=f"oT{dh}")
                   for dh in range(2)]
            sums = [att_ps.tile([P, 1], dt.float32, tag="att", name=f"sums{c}")
                    for c in range(2)]
            for j, kk in enumerate(kks):
                for col in range(2):
                    nc.tensor.matmul(sums[col],
                                     lhsT=pt[:, j, col * P:(col + 1) * P],
                                     rhs=ones_col,
                                     start=(j == 0), stop=(j == n_kk - 1))
                for dh in range(2):
                    nc.tensor.matmul(oTs[dh],
                                     lhsT=v_pieces[kk][:, dh * P:(dh + 1) * P],
                                     rhs=pt[:, j, :],
                                     start=(j == 0), stop=(j == n_kk - 1))
            sums_sb = small.tile([P, 2], dt.float32, tag="sums_sb")
            nc.vector.tensor_copy(sums_sb[:, 0:1], sums[0])
            nc.vector.tensor_copy(sums_sb[:, 1:2], sums[1])
            for col in range(2):
                nc.sync.dma_start(out=sums_d[(b + col) * P:(b + col + 1) * P],
                                  in_=sums_sb[:, col:col + 1])
            oT_sb = work.tile([P, 2, 2 * P], dt.bfloat16, tag="oTsb")
            nc.vector.tensor_copy(oT_sb[:, 0, :], oTs[0])
            nc.vector.tensor_copy(oT_sb[:, 1, :], oTs[1])

            for blk in range(2):
                osb = outp.tile([P, HID], dt.bfloat16, tag="osb")
                for nch in range(5):
                    fin = att_ps.tile([P, 512], dt.float32, tag="att", name="fin")
                    for dh in range(2):
                        nc.tensor.matmul(
                            fin,
                            lhsT=oT_sb[:, dh, blk * P:(blk + 1) * P],
                            rhs=wo_sb[:, dh, nch * 512:(nch + 1) * 512],
                            start=(dh == 0), stop=(dh == 1))
                    dst = osb[:, nch * 512:(nch + 1) * 512]
                    if nch % 2 == 0:
                        nc.scalar.copy(dst, fin)
                    else:
                        nc.vector.tensor_copy(dst, fin)
                nc.sync.dma_start(out=out[(b + blk) * P:(b + blk + 1) * P, :],
                                  in_=osb)

        # schedule: attention work for the PREVIOUS pair is emitted
        # between a tile's projection matmuls and its transposes, so the
        # PE has real work to chew on while the norm/rope chain (ACT+DVE)
        # produces the transpose inputs. Remaining const DMAs are
        # emitted just-in-time so the initial burst doesn't starve tile 0.
        pts = {}
        for m in range(NT):
            t = m // 2
            pk = proj_matmuls(m)
            if m == 0:
                for i in range(4):
                    nc.sync.dma_start(out=msk_sb[:, i, :], in_=msk[i])
            if m == 1:
                nc.sync.dma_start(out=wo_sb,
                                  in_=wo.rearrange("(g p) c -> p g c", p=P))
            if m in (2, 6, 10):
                load_quarter(m // 4 + 1)
            if m % 2 == 0:
                if t >= 1:
                    pts[t - 1] = attn_scores(t - 1)
            else:
                if t >= 1:
                    attn_out(t - 1, pts.pop(t - 1))
            norm_rope_tp(m, *pk)
        pts[NPAIR - 1] = attn_scores(NPAIR - 1)
        attn_out(NPAIR - 1, pts.pop(NPAIR - 1))

    return nc


def _host_prep(hidden_states, position_ids, cos_table, sin_table,
               Wq, Wk, Wv, Wo, q_norm_w, k_norm_w):
    f32 = np.float32
    hidden = np.asarray(hidden_states, f32).reshape(S, HID)
    pos = np.asarray(position_ids).reshape(B, S)[0].astype(np.int64)
    cos_g = np.asarray(cos_table, f32)[pos]          # [S, D]
    sin_g = np.asarray(sin_table, f32)[pos]
    qw = 1.0 + np.asarray(q_norm_w, f32)
    kw = 1.0 + np.asarray(k_norm_w, f32)

    sc = f32(SCALE)
    cq = (cos_g * qw * sc).astype(f32)
    sq = np.concatenate([-sin_g[:, :HD] * qw[HD:] * sc,
                         sin_g[:, HD:] * qw[:HD] * sc], axis=1).astype(f32)
    ck = (cos_g * kw).astype(f32)
    sk = np.concatenate([-sin_g[:, :HD] * kw[HD:],
                         sin_g[:, HD:] * kw[:HD]], axis=1).astype(f32)

    hT_t = np.ascontiguousarray(hidden.T).astype(BF16)   # [HID, S]

    jj = np.arange(P)[:, None]
    qi = np.arange(P)[None, :]
    lt = (jj <= qi).astype(f32)
    ut = (jj > qi).astype(f32)
    z = np.zeros((P, P), f32)
    o = np.ones((P, P), f32)
    masks = np.stack([
        np.concatenate([z, lt], axis=1),   # 0: kk == b+1 (rel -1)
        np.concatenate([lt, o], axis=1),   # 1: rel 0
        np.concatenate([ut, z], axis=1),   # 2: rel 4 (left ut, right dead)
        np.concatenate([o, ut], axis=1),   # 3: rel 3 (left full, right ut)
    ]).astype(BF16)

    Wq_ = np.asarray(Wq, f32)
    Wk_ = np.asarray(Wk, f32)
    Wv_ = np.asarray(Wv, f32)
    Wo_ = np.asarray(Wo, f32)

    shared = dict(hT=hT_t, cq=cq, sq=sq, ck=ck, sk=sk, msk=masks)
    in_maps = []
    for h in range(H):
        g = h // (H // KV)
        wq_h = Wq_[h * D:(h + 1) * D, :].T          # [HID, D]
        wk_g = Wk_[g * D:(g + 1) * D, :].T
        in_maps.append(dict(
            shared,
            wqk=np.ascontiguousarray(
                np.concatenate([wq_h, wk_g], axis=1)).astype(BF16),
            wv=np.ascontiguousarray(Wv_[g * D:(g + 1) * D, :].T).astype(BF16),
            wo=np.ascontiguousarray(Wo_[:, h * D:(h + 1) * D].T).astype(BF16),
        ))
    return in_maps


def get_nc():
    if "nc" not in _CACHE:
        _CACHE["nc"] = _build_nc()
    return _CACHE["nc"]


def kernel(hidden_states, position_ids, cos_table, sin_table,
           Wq, Wk, Wv, Wo, q_norm_w, k_norm_w):
    from concourse.bass_utils import run_bass_kernel_spmd

    nc = get_nc()
    in_maps = _host_prep(hidden_states, position_ids, cos_table, sin_table,
                         Wq, Wk, Wv, Wo, q_norm_w, k_norm_w)
    res = run_bass_kernel_spmd(nc, in_maps, list(range(H)))
    acc = np.zeros((S, HID), np.float32)
    for h in range(H):
        r = res.results[h]
        acc += r["out"].astype(np.float32) * (1.0 / r["sums"])[:, None]
    return acc.reshape(B, S, HID)


# revision 25
# speedup vs baseline: 1.3453x; 1.0288x over previous
"""Gemma3 sliding-window attention on 8 trn2 NeuronCores.

Sharding: tensor-parallel over the 8 query heads (1 head per core; each
core recomputes its KV head's k/v projection — no collectives). The host
pre-transposes/pre-tiles inputs into bf16; each core returns its head's
UNNORMALIZED o_proj partial [S, HID] plus per-token softmax sums; the
host applies the division and sums the 8 partials in f32.

Device kernel (identical program on all cores, different weight data):
  phase 1 (per 128-token tile): fused q|k projection (N=512) + v
  projection on PE from a resident hidden^T, RMS-norm via
  ACT-Square+accum and a DVE fast-inverse-sqrt (keeps the whole kernel
  in the `exp_and_others` ACT table set), RoPE with host-folded
  (1+w)*cos/sin*scale tables, PE transposes of q,k into [d, tok].
  phase 2 (per 256-token query pair): S^T = K Q^T on PE so the softmax
  needs no per-block transposes; exp(50*tanh(S^T/50)) with no
  max-subtraction (softcap bounds scores), 0/1 band masks, key-axis
  sums via ones-matmul, attn@V and o_proj on PE, and the final psum is
  DMA'd straight to DRAM (normalization deferred to the host).
"""

import numpy as np
import ml_dtypes

B, S, HID = 1, 2048, 2560
H, KV, D = 8, 4, 256
SCALE = 256 ** -0.5
EPS = 1e-6
P = 128
HD = D // 2          # 128, rotate_half split
NT = S // P          # 16 token tiles
KT = HID // P        # 20 contraction tiles
NPAIR = NT // 2      # 8 query-block pairs
BF16 = ml_dtypes.bfloat16

_CACHE: dict = {}


def _split_multiwait(nc):
    """walrus in this container accepts at most ONE sync wait per
    instruction; hoist extras onto wait-only EventSemaphore instructions
    inserted just before, on the same engine (same program-order
    semantics: waits are >= conditions on monotonic semaphores)."""
    import concourse.mybir as mybir

    n_new = 0
    for fn in nc.m.functions:
        for bb in fn.blocks:
            il = bb.instructions
            out = []
            for ins in il:
                si = ins.sync_info
                if si is not None and si.on_wait and len(si.on_wait) > 1:
                    waits = list(si.on_wait)
                    for w in waits[:-1]:
                        nop = mybir.InstEventSemaphore(
                            name=f"{ins.name}-hw{n_new}", ins=[], outs=[])
                        n_new += 1
                        nop.engine = ins.engine
                        nop.sync_info = mybir.SyncInfo(on_wait=[w], on_update=[])
                        nc.register_instruction(nop, overwrite=True)
                        out.append(nop)
                    ins.sync_info = mybir.SyncInfo(
                        on_wait=[waits[-1]], on_update=list(si.on_update))
                out.append(ins)
            il[:] = out


def _patch_tile_drain():
    """walrus in this container rejects multi-wait instructions; split the
    TileContext exit-drain waits into single wait_ge ops and run a
    whole-module multi-wait split pass at the very end of scheduling."""
    import concourse.mybir as mybir
    import concourse.tile as tile

    if getattr(tile.TileContext, "_drain_patched", False):
        return

    def _patched(self, tick_clock, wait_clock):
        from concourse.tile import ScopedClock

        tmp = mybir.InstNoOp(name="tmp-waits", ins=[], outs=[])
        tmp.engine = mybir.EngineType.SP
        wait_clock.add_sem_waits(tmp, ScopedClock({None: tick_clock.global_clock}))
        by_num = {h.num: h for h in self.sems.allocated().values()}
        for w in (tmp.sync_info.on_wait if tmp.sync_info else []):
            self.nc.sync.wait_ge(by_num[w.id], w.wait_value)
        self.nc.sync.drain()
        self.nc.all_engine_barrier()
        popped = self.nc._tile_sem_poison_stack.pop()
        assert popped is self._sem_poison
        self.nc.clear_and_free_semaphores(list(self.sems.allocated().values()))
        self.nc.all_engine_barrier()
        _split_multiwait(self.nc)

    tile.TileContext._drain_and_barrier = _patched
    tile.TileContext._drain_patched = True


def _build_nc():
    import concourse.bass as bass
    import concourse.mybir as mybir
    import concourse.tile as tile
    from concourse.masks import make_identity

    _patch_tile_drain()
    dt = mybir.dt
    ALU = mybir.AluOpType
    ACTF = mybir.ActivationFunctionType

    nc = bass.Bass("TRN2", target_bir_lowering=False, debug=False)

    hT = nc.dram_tensor("hT", [HID, S], dt.bfloat16, kind="ExternalInput").ap()
    wqk = nc.dram_tensor("wqk", [HID, 2 * D], dt.bfloat16, kind="ExternalInput").ap()
    wv = nc.dram_tensor("wv", [HID, D], dt.bfloat16, kind="ExternalInput").ap()
    wo = nc.dram_tensor("wo", [D, HID], dt.bfloat16, kind="ExternalInput").ap()
    cq = nc.dram_tensor("cq", [S, D], dt.float32, kind="ExternalInput").ap()
    sq = nc.dram_tensor("sq", [S, D], dt.float32, kind="ExternalInput").ap()
    ck = nc.dram_tensor("ck", [S, D], dt.float32, kind="ExternalInput").ap()
    sk = nc.dram_tensor("sk", [S, D], dt.float32, kind="ExternalInput").ap()
    msk = nc.dram_tensor("msk", [4, P, 2 * P], dt.bfloat16, kind="ExternalInput").ap()
    out = nc.dram_tensor("out", [S, HID], dt.bfloat16, kind="ExternalOutput").ap()
    sums_d = nc.dram_tensor("sums", [S], dt.float32, kind="ExternalOutput").ap()

    from contextlib import ExitStack

    with tile.TileContext(nc) as tc, ExitStack() as ctx:
        consts = ctx.enter_context(tc.tile_pool(name="consts", bufs=1))
        seq = ctx.enter_context(tc.tile_pool(name="seq", bufs=1))
        tpool = ctx.enter_context(tc.tile_pool(name="tabs", bufs=2))
        work = ctx.enter_context(tc.tile_pool(name="work", bufs=3))
        small = ctx.enter_context(tc.tile_pool(name="small", bufs=3))
        ptp = ctx.enter_context(tc.tile_pool(name="ptp", bufs=2))
        outp = ctx.enter_context(tc.tile_pool(name="outp", bufs=3))
        # PSUM budget is 8 banks; every open accumulation group needs its
        # own bank (start= marks the whole 2KB zero-region pending).
        ph1_ps = ctx.enter_context(tc.tile_pool(name="ph1_ps", bufs=2, space="PSUM"))
        st_ps = ctx.enter_context(tc.tile_pool(name="st_ps", bufs=2, space="PSUM"))
        att_ps = ctx.enter_context(tc.tile_pool(name="att_ps", bufs=4, space="PSUM"))

        # ---- constants / resident inputs ----
        # DMA emission is staggered: only what tile 0 needs goes first
        # (quarter 0 of hidden^T + weights); the rest is emitted inside
        # the main loop so the initial burst doesn't starve tile 0.
        hq = [consts.tile([P, KT, S // 8], dt.bfloat16, tag=f"hq{i}",
                          name=f"hq{i}") for i in range(8)]

        def load_eighth(i):
            nc.sync.dma_start(
                out=hq[i],
                in_=hT[:, i * (S // 8):(i + 1) * (S // 8)]
                .rearrange("(t p) n -> p t n", p=P))

        # q|k fused weights, 4 contraction-groups so early matmuls start
        # as soon as the first group lands
        wqk_sb = []
        for g in range(4):
            wt = consts.tile([P, 5, 2 * D], dt.bfloat16, tag=f"wqk{g}",
                             name=f"wqk{g}")
            nc.sync.dma_start(
                out=wt,
                in_=wqk[g * 5 * P:(g + 1) * 5 * P, :]
                .rearrange("(t p) n -> p t n", p=P))
            wqk_sb.append(wt)
        wv_sb = []
        for g in range(4):
            wt = consts.tile([P, 5, D], dt.bfloat16, tag=f"wv{g}", name=f"wv{g}")
            nc.sync.dma_start(
                out=wt,
                in_=wv[g * 5 * P:(g + 1) * 5 * P, :]
                .rearrange("(t p) n -> p t n", p=P))
            wv_sb.append(wt)
        load_eighth(0)
        wo_sb = consts.tile([P, 2, HID], dt.bfloat16, tag="wo")
        msk_sb = consts.tile([P, 4, 2 * P], dt.bfloat16, tag="msk")
        ident = consts.tile([P, P], dt.bfloat16, tag="ident")
        make_identity(nc, ident)
        ones_col = consts.tile([P, 1], dt.bfloat16, tag="ones")
        nc.gpsimd.memset(ones_col, 1.0)

        # ---- persistent per-sequence pieces (fine-grained deps) ----
        qT_pieces = [seq.tile([P, 2, 2 * P], dt.bfloat16, tag=f"qT{i}", name=f"qT{i}")
                     for i in range(NPAIR)]
        kT_pieces = [seq.tile([P, 2, 2 * P], dt.bfloat16, tag=f"kT{i}", name=f"kT{i}")
                     for i in range(NPAIR)]
        v_pieces = [seq.tile([P, D], dt.bfloat16, tag=f"v{i}", name=f"v{i}")
                    for i in range(NT)]

        def rsqrt16(ssq):
            """[P, 2] f32 sums-of-squares -> 16/sqrt(x + 256*EPS), via
            fast-inverse-sqrt bit trick + 2 Newton iterations on DVE.
            Two Halley-free Newton steps folded: the second step carries
            the x16 scale. One step leaves ~1.7e-3 rel err (fine next to
            bf16), so only one is used."""
            ms = small.tile([P, 2], dt.float32, tag="ms")
            nc.vector.tensor_scalar(ms, ssq, 256.0 * EPS, None, ALU.add)
            y = small.tile([P, 2], dt.float32, tag="y")
            yi = y.bitcast(dt.int32)
            nc.vector.tensor_scalar(yi, ms.bitcast(dt.int32), 1, None,
                                    ALU.logical_shift_right)
            nc.vector.tensor_scalar(yi, yi, -1, 0x5F3759DF, ALU.mult, ALU.add)
            t1 = small.tile([P, 2], dt.float32, tag="t1")
            nc.vector.tensor_mul(t1, y, y)
            nc.vector.tensor_mul(t1, t1, ms)
            nc.vector.tensor_scalar(t1, t1, -8.0, 24.0, ALU.mult, ALU.add)
            nc.vector.tensor_mul(y, y, t1)
            return y

        def rope(px, r, ctab, stab, dst):
            """dst (bf16) = (px*r)*ctab + shuffle(px*r)*stab, all on DVE."""
            a = work.tile([P, D], dt.float32, tag="ra")
            b = work.tile([P, D], dt.float32, tag="rb")
            nc.vector.scalar_tensor_tensor(a, px, r, ctab, ALU.mult, ALU.mult)
            nc.vector.scalar_tensor_tensor(b[:, 0:HD], px[:, HD:D], r,
                                           stab[:, 0:HD], ALU.mult, ALU.mult)
            nc.vector.scalar_tensor_tensor(b[:, HD:D], px[:, 0:HD], r,
                                           stab[:, HD:D], ALU.mult, ALU.mult)
            nc.vector.tensor_add(dst, a, b)

        def proj_matmuls(m):
            quarter = hq[m // 2]
            toff = (m % 2) * P
            cqt = tpool.tile([P, D], dt.float32, tag="cq")
            nc.sync.dma_start(out=cqt, in_=cq[m * P:(m + 1) * P, :])
            sqt = tpool.tile([P, D], dt.float32, tag="sq")
            nc.sync.dma_start(out=sqt, in_=sq[m * P:(m + 1) * P, :])
            ckt = tpool.tile([P, D], dt.float32, tag="ck")
            nc.sync.dma_start(out=ckt, in_=ck[m * P:(m + 1) * P, :])
            skt = tpool.tile([P, D], dt.float32, tag="sk")
            nc.sync.dma_start(out=skt, in_=sk[m * P:(m + 1) * P, :])

            pqk = ph1_ps.tile([P, 2 * D], dt.float32, tag="ph1", name="pqk")
            for t in range(KT):
                nc.tensor.matmul(pqk, lhsT=quarter[:, t, toff:toff + P],
                                 rhs=wqk_sb[t // 5][:, t % 5, :],
                                 start=(t == 0), stop=(t == KT - 1))
            pv = ph1_ps.tile([P, D], dt.float32, tag="ph1", name="pv")
            for t in range(KT):
                nc.tensor.matmul(pv, lhsT=quarter[:, t, toff:toff + P],
                                 rhs=wv_sb[t // 5][:, t % 5, :],
                                 start=(t == 0), stop=(t == KT - 1))
            return pqk, pv, cqt, sqt, ckt, skt

        def norm_rope_tp(m, pqk, pv, cqt, sqt, ckt, skt):
            ssq = small.tile([P, 2], dt.float32, tag="ssq")
            scr = work.tile([P, D], dt.float32, tag="scr")
            nc.scalar.activation(scr, pqk[:, 0:D], ACTF.Square,
                                 accum_out=ssq[:, 0:1])
            scr2 = work.tile([P, D], dt.float32, tag="scr")
            nc.scalar.activation(scr2, pqk[:, D:2 * D], ACTF.Square,
                                 accum_out=ssq[:, 1:2])

            r16 = rsqrt16(ssq)

            qf = work.tile([P, D], dt.bfloat16, tag="qf")
            rope(pqk[:, 0:D], r16[:, 0:1], cqt, sqt, qf)
            kf = work.tile([P, D], dt.bfloat16, tag="kf")
            rope(pqk[:, D:2 * D], r16[:, 1:2], ckt, skt, kf)
            nc.vector.tensor_copy(v_pieces[m], pv)

            pi, half = m // 2, (m % 2) * P
            for dh in range(2):
                tp = ph1_ps.tile([P, P], dt.bfloat16, tag="ph1", name="tp")
                nc.tensor.transpose(tp, qf[:, dh * HD:(dh + 1) * HD], ident)
                nc.vector.tensor_copy(qT_pieces[pi][:, dh, half:half + P], tp)
                tp2 = ph1_ps.tile([P, P], dt.bfloat16, tag="ph1", name="tp2")
                nc.tensor.transpose(tp2, kf[:, dh * HD:(dh + 1) * HD], ident)
                nc.vector.tensor_copy(kT_pieces[pi][:, dh, half:half + P], tp2)

        def pair_kks(pp):
            b = 2 * pp
            return b, list(range(max(0, b - 4), b + 2))

        def attn_scores(pp):
            """Produce the masked exp(softcap) probabilities P^T for pair
            pp into its pt tile."""
            b, kks = pair_kks(pp)
            qT = qT_pieces[pp]
            pt = ptp.tile([P, 6, 2 * P], dt.bfloat16, tag="pt", name=f"pt{pp % 2}")
            for j, kk in enumerate(kks):
                st = st_ps.tile([P, 2 * P], dt.float32, tag="st")
                for dh in range(2):
                    nc.tensor.matmul(
                        st,
                        lhsT=kT_pieces[kk // 2][:, dh, (kk % 2) * P:(kk % 2) * P + P],
                        rhs=qT[:, dh, :],
                        start=(dh == 0), stop=(dh == 1))
                th = work.tile([P, 2 * P], dt.float32, tag="th")
                nc.scalar.activation(th, st, ACTF.Tanh, scale=0.02)
                nc.scalar.activation(pt[:, j, :], th, ACTF.Exp, scale=50.0)
                rel = b - kk
                mi = {0: 1, -1: 0, 4: 2, 3: 3}.get(rel)
                if mi is not None:
                    nc.vector.tensor_mul(pt[:, j, :], pt[:, j, :], msk_sb[:, mi, :])
            return pt

        def attn_out(pp, pt):
            """Key-sums, attn@V, o_proj; unnormalized psum -> DRAM."""
            b, kks = pair_kks(pp)
            n_kk = len(kks)
            oTs = [att_ps.tile([P, 2 * P], dt.float32, tag="att", name=f"oT{dh}")
                   for dh in range(2)]
            sums = [att_ps.tile([P, 1], dt.float32, tag="att", name=f"sums{c}")
                    for c in range(2)]
            for j, kk in enumerate(kks):
                for col in range(2):
                    nc.tensor.matmul(sums[col],
                                     lhsT=pt[:, j, col * P:(col + 1) * P],
                                     rhs=ones_col,
                                     start=(j == 0), stop=(j == n_kk - 1))
                for dh in range(2):
                    nc.tensor.matmul(oTs[dh],
                                     lhsT=v_pieces[kk][:, dh * P:(dh + 1) * P],
                                     rhs=pt[:, j, :],
                                     start=(j == 0), stop=(j == n_kk - 1))
            sums_sb = small.tile([P, 2], dt.float32, tag="sums_sb")
            nc.vector.tensor_copy(sums_sb[:, 0:1], sums[0])
            nc.vector.tensor_copy(sums_sb[:, 1:2], sums[1])
            for col in range(2):
                nc.sync.dma_start(out=sums_d[(b + col) * P:(b + col + 1) * P],
                                  in_=sums_sb[:, col:col + 1])
            oT_sb = work.tile([P, 2, 2 * P], dt.bfloat16, tag="oTsb")
            nc.vector.tensor_copy(oT_sb[:, 0, :], oTs[0])
            nc.vector.tensor_copy(oT_sb[:, 1, :], oTs[1])

            for blk in range(2):
                osb = outp.tile([P, HID], dt.bfloat16, tag="osb")
                for nch in range(5):
                    fin = att_ps.tile([P, 512], dt.float32, tag="att", name="fin")
                    for dh in range(2):
                        nc.tensor.matmul(
                            fin,
                            lhsT=oT_sb[:, dh, blk * P:(blk + 1) * P],
                            rhs=wo_sb[:, dh, nch * 512:(nch + 1) * 512],
                            start=(dh == 0), stop=(dh == 1))
                    dst = osb[:, nch * 512:(nch + 1) * 512]
                    if nch % 2 == 0:
                        nc.scalar.copy(dst, fin)
                    else:
                        nc.vector.tensor_copy(dst, fin)
                nc.sync.dma_start(out=out[(b + blk) * P:(b + blk + 1) * P, :],
                                  in_=osb)

        # schedule: attention work for the PREVIOUS pair is emitted
        # between a tile's projection matmuls and its transposes, so the
        # PE has real work to chew on while the norm/rope chain (ACT+DVE)
        # produces the transpose inputs. Remaining const DMAs are
        # emitted just-in-time so the initial burst doesn't starve tile 0.
        pts = {}
        for m in range(NT):
            t = m // 2
            pk = proj_matmuls(m)
            if m == 0:
                for i in range(4):
                    nc.sync.dma_start(out=msk_sb[:, i, :], in_=msk[i])
            if m == 1:
                nc.sync.dma_start(out=wo_sb,
                                  in_=wo.rearrange("(g p) c -> p g c", p=P))
            if m % 2 == 0 and m < NT - 2:
                load_eighth(m // 2 + 1)
            if m % 2 == 0:
                if t >= 1:
                    pts[t - 1] = attn_scores(t - 1)
            else:
                if t >= 1:
                    attn_out(t - 1, pts.pop(t - 1))
            norm_rope_tp(m, *pk)
        pts[NPAIR - 1] = attn_scores(NPAIR - 1)
        attn_out(NPAIR - 1, pts.pop(NPAIR - 1))

    return nc


def _host_prep(hidden_states, position_ids, cos_table, sin_table,
               Wq, Wk, Wv, Wo, q_norm_w, k_norm_w):
    f32 = np.float32
    hidden = np.asarray(hidden_states, f32).reshape(S, HID)
    pos = np.asarray(position_ids).reshape(B, S)[0].astype(np.int64)
    cos_g = np.asarray(cos_table, f32)[pos]          # [S, D]
    sin_g = np.asarray(sin_table, f32)[pos]
    qw = 1.0 + np.asarray(q_norm_w, f32)
    kw = 1.0 + np.asarray(k_norm_w, f32)

    sc = f32(SCALE)
    cq = (cos_g * qw * sc).astype(f32)
    sq = np.concatenate([-sin_g[:, :HD] * qw[HD:] * sc,
                         sin_g[:, HD:] * qw[:HD] * sc], axis=1).astype(f32)
    ck = (cos_g * kw).astype(f32)
    sk = np.concatenate([-sin_g[:, :HD] * kw[HD:],
                         sin_g[:, HD:] * kw[:HD]], axis=1).astype(f32)

    hT_t = np.ascontiguousarray(hidden.T).astype(BF16)   # [HID, S]

    jj = np.arange(P)[:, None]
    qi = np.arange(P)[None, :]
    lt = (jj <= qi).astype(f32)
    ut = (jj > qi).astype(f32)
    z = np.zeros((P, P), f32)
    o = np.ones((P, P), f32)
    masks = np.stack([
        np.concatenate([z, lt], axis=1),   # 0: kk == b+1 (rel -1)
        np.concatenate([lt, o], axis=1),   # 1: rel 0
        np.concatenate([ut, z], axis=1),   # 2: rel 4 (left ut, right dead)
        np.concatenate([o, ut], axis=1),   # 3: rel 3 (left full, right ut)
    ]).astype(BF16)

    Wq_ = np.asarray(Wq, f32)
    Wk_ = np.asarray(Wk, f32)
    Wv_ = np.asarray(Wv, f32)
    Wo_ = np.asarray(Wo, f32)

    shared = dict(hT=hT_t, cq=cq, sq=sq, ck=ck, sk=sk, msk=masks)
    in_maps = []
    for h in range(H):
        g = h // (H // KV)
        wq_h = Wq_[h * D:(h + 1) * D, :].T          # [HID, D]
        wk_g = Wk_[g * D:(g + 1) * D, :].T
        in_maps.append(dict(
            shared,
            wqk=np.ascontiguousarray(
                np.concatenate([wq_h, wk_g], axis=1)).astype(BF16),
            wv=np.ascontiguousarray(Wv_[g * D:(g + 1) * D, :].T).astype(BF16),
            wo=np.ascontiguousarray(Wo_[:, h * D:(h + 1) * D].T).astype(BF16),
        ))
    return in_maps


def get_nc():
    if "nc" not in _CACHE:
        _CACHE["nc"] = _build_nc()
    return _CACHE["nc"]


def kernel(hidden_states, position_ids, cos_table, sin_table,
           Wq, Wk, Wv, Wo, q_norm_w, k_norm_w):
    from concourse.bass_utils import run_bass_kernel_spmd

    nc = get_nc()
    in_maps = _host_prep(hidden_states, position_ids, cos_table, sin_table,
                         Wq, Wk, Wv, Wo, q_norm_w, k_norm_w)
    res = run_bass_kernel_spmd(nc, in_maps, list(range(H)))
    acc = np.zeros((S, HID), np.float32)
    for h in range(H):
        r = res.results[h]
        acc += r["out"].astype(np.float32) * (1.0 / r["sums"])[:, None]
    return acc.reshape(B, S, HID)


# revision 26
# speedup vs baseline: 1.3532x; 1.0059x over previous
"""Gemma3 sliding-window attention on 8 trn2 NeuronCores.

Sharding: tensor-parallel over the 8 query heads (1 head per core; each
core recomputes its KV head's k/v projection — no collectives). The host
pre-transposes/pre-tiles inputs into bf16; each core returns its head's
UNNORMALIZED o_proj partial [S, HID] plus per-token softmax sums; the
host applies the division and sums the 8 partials in f32.

Device kernel (identical program on all cores, different weight data):
  phase 1 (per 128-token tile): fused q|k projection (N=512) + v
  projection on PE from a resident hidden^T, RMS-norm via
  ACT-Square+accum and a DVE fast-inverse-sqrt (keeps the whole kernel
  in the `exp_and_others` ACT table set), RoPE with host-folded
  (1+w)*cos/sin*scale tables, PE transposes of q,k into [d, tok].
  phase 2 (per 256-token query pair): S^T = K Q^T on PE so the softmax
  needs no per-block transposes; exp(50*tanh(S^T/50)) with no
  max-subtraction (softcap bounds scores), 0/1 band masks, key-axis
  sums via ones-matmul, attn@V and o_proj on PE, and the final psum is
  DMA'd straight to DRAM (normalization deferred to the host).
"""

import numpy as np
import ml_dtypes

B, S, HID = 1, 2048, 2560
H, KV, D = 8, 4, 256
SCALE = 256 ** -0.5
EPS = 1e-6
P = 128
HD = D // 2          # 128, rotate_half split
NT = S // P          # 16 token tiles
KT = HID // P        # 20 contraction tiles
NPAIR = NT // 2      # 8 query-block pairs
BF16 = ml_dtypes.bfloat16

_CACHE: dict = {}


def _split_multiwait(nc):
    """walrus in this container accepts at most ONE sync wait per
    instruction; hoist extras onto wait-only EventSemaphore instructions
    inserted just before, on the same engine (same program-order
    semantics: waits are >= conditions on monotonic semaphores)."""
    import concourse.mybir as mybir

    n_new = 0
    for fn in nc.m.functions:
        for bb in fn.blocks:
            il = bb.instructions
            out = []
            for ins in il:
                si = ins.sync_info
                if si is not None and si.on_wait and len(si.on_wait) > 1:
                    waits = list(si.on_wait)
                    for w in waits[:-1]:
                        nop = mybir.InstEventSemaphore(
                            name=f"{ins.name}-hw{n_new}", ins=[], outs=[])
                        n_new += 1
                        nop.engine = ins.engine
                        nop.sync_info = mybir.SyncInfo(on_wait=[w], on_update=[])
                        nc.register_instruction(nop, overwrite=True)
                        out.append(nop)
                    ins.sync_info = mybir.SyncInfo(
                        on_wait=[waits[-1]], on_update=list(si.on_update))
                out.append(ins)
            il[:] = out


def _patch_tile_drain():
    """walrus in this container rejects multi-wait instructions; split the
    TileContext exit-drain waits into single wait_ge ops and run a
    whole-module multi-wait split pass at the very end of scheduling."""
    import concourse.mybir as mybir
    import concourse.tile as tile

    if getattr(tile.TileContext, "_drain_patched", False):
        return

    def _patched(self, tick_clock, wait_clock):
        from concourse.tile import ScopedClock

        tmp = mybir.InstNoOp(name="tmp-waits", ins=[], outs=[])
        tmp.engine = mybir.EngineType.SP
        wait_clock.add_sem_waits(tmp, ScopedClock({None: tick_clock.global_clock}))
        by_num = {h.num: h for h in self.sems.allocated().values()}
        for w in (tmp.sync_info.on_wait if tmp.sync_info else []):
            self.nc.sync.wait_ge(by_num[w.id], w.wait_value)
        self.nc.sync.drain()
        self.nc.all_engine_barrier()
        popped = self.nc._tile_sem_poison_stack.pop()
        assert popped is self._sem_poison
        self.nc.clear_and_free_semaphores(list(self.sems.allocated().values()))
        self.nc.all_engine_barrier()
        _split_multiwait(self.nc)

    tile.TileContext._drain_and_barrier = _patched
    tile.TileContext._drain_patched = True


def _build_nc():
    import concourse.bass as bass
    import concourse.mybir as mybir
    import concourse.tile as tile
    from concourse.masks import make_identity

    _patch_tile_drain()
    dt = mybir.dt
    ALU = mybir.AluOpType
    ACTF = mybir.ActivationFunctionType

    nc = bass.Bass("TRN2", target_bir_lowering=False, debug=False)

    hT = nc.dram_tensor("hT", [HID, S], dt.bfloat16, kind="ExternalInput").ap()
    wqk = nc.dram_tensor("wqk", [HID, 2 * D], dt.bfloat16, kind="ExternalInput").ap()
    wv = nc.dram_tensor("wv", [HID, D], dt.bfloat16, kind="ExternalInput").ap()
    wo = nc.dram_tensor("wo", [D, HID], dt.bfloat16, kind="ExternalInput").ap()
    cq = nc.dram_tensor("cq", [S, D], dt.float32, kind="ExternalInput").ap()
    sq = nc.dram_tensor("sq", [S, D], dt.float32, kind="ExternalInput").ap()
    ck = nc.dram_tensor("ck", [S, D], dt.float32, kind="ExternalInput").ap()
    sk = nc.dram_tensor("sk", [S, D], dt.float32, kind="ExternalInput").ap()
    msk = nc.dram_tensor("msk", [4, P, 2 * P], dt.bfloat16, kind="ExternalInput").ap()
    out = nc.dram_tensor("out", [S, HID], dt.bfloat16, kind="ExternalOutput").ap()
    sums_d = nc.dram_tensor("sums", [S], dt.float32, kind="ExternalOutput").ap()

    from contextlib import ExitStack

    with tile.TileContext(nc) as tc, ExitStack() as ctx:
        consts = ctx.enter_context(tc.tile_pool(name="consts", bufs=1))
        seq = ctx.enter_context(tc.tile_pool(name="seq", bufs=1))
        tpool = ctx.enter_context(tc.tile_pool(name="tabs", bufs=2))
        work = ctx.enter_context(tc.tile_pool(name="work", bufs=3))
        small = ctx.enter_context(tc.tile_pool(name="small", bufs=3))
        ptp = ctx.enter_context(tc.tile_pool(name="ptp", bufs=2))
        outp = ctx.enter_context(tc.tile_pool(name="outp", bufs=3))
        # PSUM budget is 8 banks; every open accumulation group needs its
        # own bank (start= marks the whole 2KB zero-region pending).
        ph1_ps = ctx.enter_context(tc.tile_pool(name="ph1_ps", bufs=2, space="PSUM"))
        st_ps = ctx.enter_context(tc.tile_pool(name="st_ps", bufs=2, space="PSUM"))
        att_ps = ctx.enter_context(tc.tile_pool(name="att_ps", bufs=4, space="PSUM"))

        # ---- constants / resident inputs ----
        # DMA emission is staggered: only what tile 0 needs goes first
        # (quarter 0 of hidden^T + weights); the rest is emitted inside
        # the main loop so the initial burst doesn't starve tile 0.
        hq = [consts.tile([P, KT, S // 8], dt.bfloat16, tag=f"hq{i}",
                          name=f"hq{i}") for i in range(8)]

        def load_eighth(i):
            nc.sync.dma_start(
                out=hq[i],
                in_=hT[:, i * (S // 8):(i + 1) * (S // 8)]
                .rearrange("(t p) n -> p t n", p=P))

        # q|k fused weights, 4 contraction-groups so early matmuls start
        # as soon as the first group lands. DMA order: first hidden
        # eighth, then weight groups in contraction order (what tile 0's
        # matmul sequence consumes first).
        load_eighth(0)
        wqk_sb = [consts.tile([P, 5, 2 * D], dt.bfloat16, tag=f"wqk{g}",
                              name=f"wqk{g}") for g in range(4)]
        wv_sb = [consts.tile([P, 5, D], dt.bfloat16, tag=f"wv{g}",
                             name=f"wv{g}") for g in range(4)]
        for g in range(4):
            nc.sync.dma_start(
                out=wqk_sb[g],
                in_=wqk[g * 5 * P:(g + 1) * 5 * P, :]
                .rearrange("(t p) n -> p t n", p=P))
            nc.sync.dma_start(
                out=wv_sb[g],
                in_=wv[g * 5 * P:(g + 1) * 5 * P, :]
                .rearrange("(t p) n -> p t n", p=P))
        wo_sb = consts.tile([P, 2, HID], dt.bfloat16, tag="wo")
        msk_sb = consts.tile([P, 4, 2 * P], dt.bfloat16, tag="msk")
        ident = consts.tile([P, P], dt.bfloat16, tag="ident")
        make_identity(nc, ident)
        ones_col = consts.tile([P, 1], dt.bfloat16, tag="ones")
        nc.gpsimd.memset(ones_col, 1.0)

        # ---- persistent per-sequence pieces (fine-grained deps) ----
        qT_pieces = [seq.tile([P, 2, 2 * P], dt.bfloat16, tag=f"qT{i}", name=f"qT{i}")
                     for i in range(NPAIR)]
        kT_pieces = [seq.tile([P, 2, 2 * P], dt.bfloat16, tag=f"kT{i}", name=f"kT{i}")
                     for i in range(NPAIR)]
        v_pieces = [seq.tile([P, D], dt.bfloat16, tag=f"v{i}", name=f"v{i}")
                    for i in range(NT)]

        def rsqrt16(ssq):
            """[P, 2] f32 sums-of-squares -> 16/sqrt(x + 256*EPS), via
            fast-inverse-sqrt bit trick + 2 Newton iterations on DVE.
            Two Halley-free Newton steps folded: the second step carries
            the x16 scale. One step leaves ~1.7e-3 rel err (fine next to
            bf16), so only one is used."""
            ms = small.tile([P, 2], dt.float32, tag="ms")
            nc.vector.tensor_scalar(ms, ssq, 256.0 * EPS, None, ALU.add)
            y = small.tile([P, 2], dt.float32, tag="y")
            yi = y.bitcast(dt.int32)
            nc.vector.tensor_scalar(yi, ms.bitcast(dt.int32), 1, None,
                                    ALU.logical_shift_right)
            nc.vector.tensor_scalar(yi, yi, -1, 0x5F3759DF, ALU.mult, ALU.add)
            t1 = small.tile([P, 2], dt.float32, tag="t1")
            nc.vector.tensor_mul(t1, y, y)
            nc.vector.tensor_mul(t1, t1, ms)
            nc.vector.tensor_scalar(t1, t1, -8.0, 24.0, ALU.mult, ALU.add)
            nc.vector.tensor_mul(y, y, t1)
            return y

        def rope(px, r, ctab, stab, dst):
            """dst (bf16) = (px*r)*ctab + shuffle(px*r)*stab, all on DVE."""
            a = work.tile([P, D], dt.float32, tag="ra")
            b = work.tile([P, D], dt.float32, tag="rb")
            nc.vector.scalar_tensor_tensor(a, px, r, ctab, ALU.mult, ALU.mult)
            nc.vector.scalar_tensor_tensor(b[:, 0:HD], px[:, HD:D], r,
                                           stab[:, 0:HD], ALU.mult, ALU.mult)
            nc.vector.scalar_tensor_tensor(b[:, HD:D], px[:, 0:HD], r,
                                           stab[:, HD:D], ALU.mult, ALU.mult)
            nc.vector.tensor_add(dst, a, b)

        def proj_matmuls(m):
            quarter = hq[m // 2]
            toff = (m % 2) * P
            cqt = tpool.tile([P, D], dt.float32, tag="cq")
            nc.sync.dma_start(out=cqt, in_=cq[m * P:(m + 1) * P, :])
            sqt = tpool.tile([P, D], dt.float32, tag="sq")
            nc.sync.dma_start(out=sqt, in_=sq[m * P:(m + 1) * P, :])
            ckt = tpool.tile([P, D], dt.float32, tag="ck")
            nc.sync.dma_start(out=ckt, in_=ck[m * P:(m + 1) * P, :])
            skt = tpool.tile([P, D], dt.float32, tag="sk")
            nc.sync.dma_start(out=skt, in_=sk[m * P:(m + 1) * P, :])

            pqk = ph1_ps.tile([P, 2 * D], dt.float32, tag="ph1", name="pqk")
            for t in range(KT):
                nc.tensor.matmul(pqk, lhsT=quarter[:, t, toff:toff + P],
                                 rhs=wqk_sb[t // 5][:, t % 5, :],
                                 start=(t == 0), stop=(t == KT - 1))
            pv = ph1_ps.tile([P, D], dt.float32, tag="ph1", name="pv")
            for t in range(KT):
                nc.tensor.matmul(pv, lhsT=quarter[:, t, toff:toff + P],
                                 rhs=wv_sb[t // 5][:, t % 5, :],
                                 start=(t == 0), stop=(t == KT - 1))
            return pqk, pv, cqt, sqt, ckt, skt

        def norm_rope_tp(m, pqk, pv, cqt, sqt, ckt, skt):
            ssq = small.tile([P, 2], dt.float32, tag="ssq")
            scr = work.tile([P, D], dt.float32, tag="scr")
            nc.scalar.activation(scr, pqk[:, 0:D], ACTF.Square,
                                 accum_out=ssq[:, 0:1])
            scr2 = work.tile([P, D], dt.float32, tag="scr")
            nc.scalar.activation(scr2, pqk[:, D:2 * D], ACTF.Square,
                                 accum_out=ssq[:, 1:2])

            r16 = rsqrt16(ssq)

            qf = work.tile([P, D], dt.bfloat16, tag="qf")
            rope(pqk[:, 0:D], r16[:, 0:1], cqt, sqt, qf)
            kf = work.tile([P, D], dt.bfloat16, tag="kf")
            rope(pqk[:, D:2 * D], r16[:, 1:2], ckt, skt, kf)
            nc.vector.tensor_copy(v_pieces[m], pv)

            pi, half = m // 2, (m % 2) * P
            for dh in range(2):
                tp = ph1_ps.tile([P, P], dt.bfloat16, tag="ph1", name="tp")
                nc.tensor.transpose(tp, qf[:, dh * HD:(dh + 1) * HD], ident)
                nc.vector.tensor_copy(qT_pieces[pi][:, dh, half:half + P], tp)
                tp2 = ph1_ps.tile([P, P], dt.bfloat16, tag="ph1", name="tp2")
                nc.tensor.transpose(tp2, kf[:, dh * HD:(dh + 1) * HD], ident)
                nc.vector.tensor_copy(kT_pieces[pi][:, dh, half:half + P], tp2)

        def pair_kks(pp):
            b = 2 * pp
            return b, list(range(max(0, b - 4), b + 2))

        def attn_scores(pp):
            """Produce the masked exp(softcap) probabilities P^T for pair
            pp into its pt tile."""
            b, kks = pair_kks(pp)
            qT = qT_pieces[pp]
            pt = ptp.tile([P, 6, 2 * P], dt.bfloat16, tag="pt", name=f"pt{pp % 2}")
            for j, kk in enumerate(kks):
                st = st_ps.tile([P, 2 * P], dt.float32, tag="st")
                for dh in range(2):
                    nc.tensor.matmul(
                        st,
                        lhsT=kT_pieces[kk // 2][:, dh, (kk % 2) * P:(kk % 2) * P + P],
                        rhs=qT[:, dh, :],
                        start=(dh == 0), stop=(dh == 1))
                th = work.tile([P, 2 * P], dt.float32, tag="th")
                nc.scalar.activation(th, st, ACTF.Tanh, scale=0.02)
                nc.scalar.activation(pt[:, j, :], th, ACTF.Exp, scale=50.0)
                rel = b - kk
                mi = {0: 1, -1: 0, 4: 2, 3: 3}.get(rel)
                if mi is not None:
                    nc.vector.tensor_mul(pt[:, j, :], pt[:, j, :], msk_sb[:, mi, :])
            return pt

        def attn_out(pp, pt):
            """Key-sums, attn@V, o_proj; unnormalized psum -> DRAM."""
            b, kks = pair_kks(pp)
            n_kk = len(kks)
            oTs = [att_ps.tile([P, 2 * P], dt.float32, tag="att", name=f"oT{dh}")
                   for dh in range(2)]
            sums = [att_ps.tile([P, 1], dt.float32, tag="att", name=f"sums{c}")
                    for c in range(2)]
            for j, kk in enumerate(kks):
                for col in range(2):
                    nc.tensor.matmul(sums[col],
                                     lhsT=pt[:, j, col * P:(col + 1) * P],
                                     rhs=ones_col,
                                     start=(j == 0), stop=(j == n_kk - 1))
                for dh in range(2):
                    nc.tensor.matmul(oTs[dh],
                                     lhsT=v_pieces[kk][:, dh * P:(dh + 1) * P],
                                     rhs=pt[:, j, :],
                                     start=(j == 0), stop=(j == n_kk - 1))
            sums_sb = small.tile([P, 2], dt.float32, tag="sums_sb")
            nc.vector.tensor_copy(sums_sb[:, 0:1], sums[0])
            nc.vector.tensor_copy(sums_sb[:, 1:2], sums[1])
            for col in range(2):
                nc.sync.dma_start(out=sums_d[(b + col) * P:(b + col + 1) * P],
                                  in_=sums_sb[:, col:col + 1])
            oT_sb = work.tile([P, 2, 2 * P], dt.bfloat16, tag="oTsb")
            nc.vector.tensor_copy(oT_sb[:, 0, :], oTs[0])
            nc.vector.tensor_copy(oT_sb[:, 1, :], oTs[1])

            for blk in range(2):
                osb = outp.tile([P, HID], dt.bfloat16, tag="osb")
                for nch in range(5):
                    fin = att_ps.tile([P, 512], dt.float32, tag="att", name="fin")
                    for dh in range(2):
                        nc.tensor.matmul(
                            fin,
                            lhsT=oT_sb[:, dh, blk * P:(blk + 1) * P],
                            rhs=wo_sb[:, dh, nch * 512:(nch + 1) * 512],
                            start=(dh == 0), stop=(dh == 1))
                    dst = osb[:, nch * 512:(nch + 1) * 512]
                    if nch % 2 == 0:
                        nc.scalar.copy(dst, fin)
                    else:
                        nc.vector.tensor_copy(dst, fin)
                nc.sync.dma_start(out=out[(b + blk) * P:(b + blk + 1) * P, :],
                                  in_=osb)

        # schedule: attention work for the PREVIOUS pair is emitted
        # between a tile's projection matmuls and its transposes, so the
        # PE has real work to chew on while the norm/rope chain (ACT+DVE)
        # produces the transpose inputs. Remaining const DMAs are
        # emitted just-in-time so the initial burst doesn't starve tile 0.
        pts = {}
        for m in range(NT):
            t = m // 2
            pk = proj_matmuls(m)
            if m == 0:
                for i in range(4):
                    nc.sync.dma_start(out=msk_sb[:, i, :], in_=msk[i])
            if m == 1:
                nc.sync.dma_start(out=wo_sb,
                                  in_=wo.rearrange("(g p) c -> p g c", p=P))
            if m % 2 == 0 and m < NT - 2:
                load_eighth(m // 2 + 1)
            if m % 2 == 0:
                if t >= 1:
                    pts[t - 1] = attn_scores(t - 1)
            else:
                if t >= 1:
                    attn_out(t - 1, pts.pop(t - 1))
            norm_rope_tp(m, *pk)
        pts[NPAIR - 1] = attn_scores(NPAIR - 1)
        attn_out(NPAIR - 1, pts.pop(NPAIR - 1))

    return nc


def _host_prep(hidden_states, position_ids, cos_table, sin_table,
               Wq, Wk, Wv, Wo, q_norm_w, k_norm_w):
    f32 = np.float32
    hidden = np.asarray(hidden_states, f32).reshape(S, HID)
    pos = np.asarray(position_ids).reshape(B, S)[0].astype(np.int64)
    cos_g = np.asarray(cos_table, f32)[pos]          # [S, D]
    sin_g = np.asarray(sin_table, f32)[pos]
    qw = 1.0 + np.asarray(q_norm_w, f32)
    kw = 1.0 + np.asarray(k_norm_w, f32)

    sc = f32(SCALE)
    cq = (cos_g * qw * sc).astype(f32)
    sq = np.concatenate([-sin_g[:, :HD] * qw[HD:] * sc,
                         sin_g[:, HD:] * qw[:HD] * sc], axis=1).astype(f32)
    ck = (cos_g * kw).astype(f32)
    sk = np.concatenate([-sin_g[:, :HD] * kw[HD:],
                         sin_g[:, HD:] * kw[:HD]], axis=1).astype(f32)

    hT_t = np.ascontiguousarray(hidden.T).astype(BF16)   # [HID, S]

    jj = np.arange(P)[:, None]
    qi = np.arange(P)[None, :]
    lt = (jj <= qi).astype(f32)
    ut = (jj > qi).astype(f32)
    z = np.zeros((P, P), f32)
    o = np.ones((P, P), f32)
    masks = np.stack([
        np.concatenate([z, lt], axis=1),   # 0: kk == b+1 (rel -1)
        np.concatenate([lt, o], axis=1),   # 1: rel 0
        np.concatenate([ut, z], axis=1),   # 2: rel 4 (left ut, right dead)
        np.concatenate([o, ut], axis=1),   # 3: rel 3 (left full, right ut)
    ]).astype(BF16)

    Wq_ = np.asarray(Wq, f32)
    Wk_ = np.asarray(Wk, f32)
    Wv_ = np.asarray(Wv, f32)
    Wo_ = np.asarray(Wo, f32)

    shared = dict(hT=hT_t, cq=cq, sq=sq, ck=ck, sk=sk, msk=masks)
    in_maps = []
    for h in range(H):
        g = h // (H // KV)
        wq_h = Wq_[h * D:(h + 1) * D, :].T          # [HID, D]
        wk_g = Wk_[g * D:(g + 1) * D, :].T
        in_maps.append(dict(
            shared,
            wqk=np.ascontiguousarray(
                np.concatenate([wq_h, wk_g], axis=1)).astype(BF16),
            wv=np.ascontiguousarray(Wv_[g * D:(g + 1) * D, :].T).astype(BF16),
            wo=np.ascontiguousarray(Wo_[:, h * D:(h + 1) * D].T).astype(BF16),
        ))
    return in_maps


def get_nc():
    if "nc" not in _CACHE:
        _CACHE["nc"] = _build_nc()
    return _CACHE["nc"]


def kernel(hidden_states, position_ids, cos_table, sin_table,
           Wq, Wk, Wv, Wo, q_norm_w, k_norm_w):
    from concourse.bass_utils import run_bass_kernel_spmd

    nc = get_nc()
    in_maps = _host_prep(hidden_states, position_ids, cos_table, sin_table,
                         Wq, Wk, Wv, Wo, q_norm_w, k_norm_w)
    res = run_bass_kernel_spmd(nc, in_maps, list(range(H)))
    acc = np.zeros((S, HID), np.float32)
    for h in range(H):
        r = res.results[h]
        acc += r["out"].astype(np.float32) * (1.0 / r["sums"])[:, None]
    return acc.reshape(B, S, HID)


# revision 29
# speedup vs baseline: 1.3922x; 1.0288x over previous
"""Gemma3 sliding-window attention on 8 trn2 NeuronCores.

Sharding: tensor-parallel over the 8 query heads (1 head per core; each
core recomputes its KV head's k/v projection — no collectives). The host
pre-transposes/pre-tiles inputs into bf16; each core returns its head's
UNNORMALIZED o_proj partial [S, HID] plus per-token softmax sums; the
host applies the division and sums the 8 partials in f32.

Device kernel (identical program on all cores, different weight data):
  phase 1 (per 128-token tile): fused q|k projection (N=512) + v
  projection on PE from a resident hidden^T, RMS-norm via
  ACT-Square+accum and a DVE fast-inverse-sqrt (keeps the whole kernel
  in the `exp_and_others` ACT table set), RoPE with host-folded
  (1+w)*cos/sin*scale tables, PE transposes of q,k into [d, tok].
  phase 2 (per 256-token query pair): S^T = K Q^T on PE so the softmax
  needs no per-block transposes; exp(50*tanh(S^T/50)) with no
  max-subtraction (softcap bounds scores), 0/1 band masks, key-axis
  sums via ones-matmul, attn@V and o_proj on PE, and the final psum is
  DMA'd straight to DRAM (normalization deferred to the host).
"""

import numpy as np
import ml_dtypes

B, S, HID = 1, 2048, 2560
H, KV, D = 8, 4, 256
SCALE = 256 ** -0.5
EPS = 1e-6
P = 128
HD = D // 2          # 128, rotate_half split
NT = S // P          # 16 token tiles
KT = HID // P        # 20 contraction tiles
NPAIR = NT // 2      # 8 query-block pairs
BF16 = ml_dtypes.bfloat16

_CACHE: dict = {}


def _split_multiwait(nc):
    """walrus in this container accepts at most ONE sync wait per
    instruction; hoist extras onto wait-only EventSemaphore instructions
    inserted just before, on the same engine (same program-order
    semantics: waits are >= conditions on monotonic semaphores)."""
    import concourse.mybir as mybir

    n_new = 0
    for fn in nc.m.functions:
        for bb in fn.blocks:
            il = bb.instructions
            out = []
            for ins in il:
                si = ins.sync_info
                if si is not None and si.on_wait and len(si.on_wait) > 1:
                    waits = list(si.on_wait)
                    for w in waits[:-1]:
                        nop = mybir.InstEventSemaphore(
                            name=f"{ins.name}-hw{n_new}", ins=[], outs=[])
                        n_new += 1
                        nop.engine = ins.engine
                        nop.sync_info = mybir.SyncInfo(on_wait=[w], on_update=[])
                        nc.register_instruction(nop, overwrite=True)
                        out.append(nop)
                    ins.sync_info = mybir.SyncInfo(
                        on_wait=[waits[-1]], on_update=list(si.on_update))
                out.append(ins)
            il[:] = out


def _patch_tile_drain():
    """walrus in this container rejects multi-wait instructions; split the
    TileContext exit-drain waits into single wait_ge ops and run a
    whole-module multi-wait split pass at the very end of scheduling."""
    import concourse.mybir as mybir
    import concourse.tile as tile

    if getattr(tile.TileContext, "_drain_patched", False):
        return

    def _patched(self, tick_clock, wait_clock):
        from concourse.tile import ScopedClock

        tmp = mybir.InstNoOp(name="tmp-waits", ins=[], outs=[])
        tmp.engine = mybir.EngineType.SP
        wait_clock.add_sem_waits(tmp, ScopedClock({None: tick_clock.global_clock}))
        by_num = {h.num: h for h in self.sems.allocated().values()}
        for w in (tmp.sync_info.on_wait if tmp.sync_info else []):
            self.nc.sync.wait_ge(by_num[w.id], w.wait_value)
        self.nc.sync.drain()
        self.nc.all_engine_barrier()
        popped = self.nc._tile_sem_poison_stack.pop()
        assert popped is self._sem_poison
        self.nc.clear_and_free_semaphores(list(self.sems.allocated().values()))
        self.nc.all_engine_barrier(sem_only=True)
        _split_multiwait(self.nc)

    tile.TileContext._drain_and_barrier = _patched
    tile.TileContext._drain_patched = True


def _build_nc():
    import concourse.bass as bass
    import concourse.mybir as mybir
    import concourse.tile as tile
    from concourse.masks import make_identity

    _patch_tile_drain()
    dt = mybir.dt
    ALU = mybir.AluOpType
    ACTF = mybir.ActivationFunctionType

    nc = bass.Bass("TRN2", target_bir_lowering=False, debug=False)

    hT = nc.dram_tensor("hT", [HID, S], dt.bfloat16, kind="ExternalInput").ap()
    wqk = nc.dram_tensor("wqk", [HID, 2 * D], dt.bfloat16, kind="ExternalInput").ap()
    wv = nc.dram_tensor("wv", [HID, D], dt.bfloat16, kind="ExternalInput").ap()
    wo = nc.dram_tensor("wo", [D, HID], dt.bfloat16, kind="ExternalInput").ap()
    cq = nc.dram_tensor("cq", [S, D], dt.float32, kind="ExternalInput").ap()
    sq = nc.dram_tensor("sq", [S, D], dt.float32, kind="ExternalInput").ap()
    ck = nc.dram_tensor("ck", [S, D], dt.float32, kind="ExternalInput").ap()
    sk = nc.dram_tensor("sk", [S, D], dt.float32, kind="ExternalInput").ap()
    msk = nc.dram_tensor("msk", [4, P, 2 * P], dt.bfloat16, kind="ExternalInput").ap()
    out = nc.dram_tensor("out", [S, HID], dt.bfloat16, kind="ExternalOutput").ap()
    sums_d = nc.dram_tensor("sums", [S], dt.float32, kind="ExternalOutput").ap()

    from contextlib import ExitStack

    with tile.TileContext(nc) as tc, ExitStack() as ctx:
        consts = ctx.enter_context(tc.tile_pool(name="consts", bufs=1))
        seq = ctx.enter_context(tc.tile_pool(name="seq", bufs=1))
        tpool = ctx.enter_context(tc.tile_pool(name="tabs", bufs=2))
        work = ctx.enter_context(tc.tile_pool(name="work", bufs=3))
        small = ctx.enter_context(tc.tile_pool(name="small", bufs=3))
        ptp = ctx.enter_context(tc.tile_pool(name="ptp", bufs=2))
        outp = ctx.enter_context(tc.tile_pool(name="outp", bufs=3))
        # PSUM budget is 8 banks; every open accumulation group needs its
        # own bank (start= marks the whole 2KB zero-region pending).
        ph1_ps = ctx.enter_context(tc.tile_pool(name="ph1_ps", bufs=2, space="PSUM"))
        st_ps = ctx.enter_context(tc.tile_pool(name="st_ps", bufs=2, space="PSUM"))
        att_ps = ctx.enter_context(tc.tile_pool(name="att_ps", bufs=4, space="PSUM"))

        # ---- constants / resident inputs ----
        # DMA emission is staggered: only what tile 0 needs goes first
        # (quarter 0 of hidden^T + weights); the rest is emitted inside
        # the main loop so the initial burst doesn't starve tile 0.
        hq = [consts.tile([P, KT, S // 8], dt.bfloat16, tag=f"hq{i}",
                          name=f"hq{i}") for i in range(8)]

        def load_eighth(i):
            nc.sync.dma_start(
                out=hq[i],
                in_=hT[:, i * (S // 8):(i + 1) * (S // 8)]
                .rearrange("(t p) n -> p t n", p=P))

        # q|k fused weights, 4 contraction-groups so early matmuls start
        # as soon as the first group lands. DMA order: first hidden
        # eighth, then weight groups in contraction order (what tile 0's
        # matmul sequence consumes first).
        load_eighth(0)
        wqk_sb = [consts.tile([P, 5, 2 * D], dt.bfloat16, tag=f"wqk{g}",
                              name=f"wqk{g}") for g in range(4)]
        wv_sb = [consts.tile([P, 5, D], dt.bfloat16, tag=f"wv{g}",
                             name=f"wv{g}") for g in range(4)]
        for g in range(4):
            nc.sync.dma_start(
                out=wqk_sb[g],
                in_=wqk[g * 5 * P:(g + 1) * 5 * P, :]
                .rearrange("(t p) n -> p t n", p=P))
            nc.sync.dma_start(
                out=wv_sb[g],
                in_=wv[g * 5 * P:(g + 1) * 5 * P, :]
                .rearrange("(t p) n -> p t n", p=P))
        wo_sb = consts.tile([P, 2, HID], dt.bfloat16, tag="wo")
        msk_sb = consts.tile([P, 4, 2 * P], dt.bfloat16, tag="msk")
        ident = consts.tile([P, P], dt.bfloat16, tag="ident")
        make_identity(nc, ident)
        ones_col = consts.tile([P, 1], dt.bfloat16, tag="ones")
        nc.gpsimd.memset(ones_col, 1.0)

        # ---- persistent per-sequence pieces (fine-grained deps) ----
        qT_pieces = [seq.tile([P, 2, 2 * P], dt.bfloat16, tag=f"qT{i}", name=f"qT{i}")
                     for i in range(NPAIR)]
        kT_pieces = [seq.tile([P, 2, 2 * P], dt.bfloat16, tag=f"kT{i}", name=f"kT{i}")
                     for i in range(NPAIR)]
        v_pieces = [seq.tile([P, D], dt.bfloat16, tag=f"v{i}", name=f"v{i}")
                    for i in range(NT)]

        def rsqrt16(ssq):
            """[P, 2] f32 sums-of-squares -> 16/sqrt(x + 256*EPS), via
            fast-inverse-sqrt bit trick + 2 Newton iterations on DVE.
            Two Halley-free Newton steps folded: the second step carries
            the x16 scale. One step leaves ~1.7e-3 rel err (fine next to
            bf16), so only one is used."""
            ms = small.tile([P, 2], dt.float32, tag="ms")
            nc.vector.tensor_scalar(ms, ssq, 256.0 * EPS, None, ALU.add)
            y = small.tile([P, 2], dt.float32, tag="y")
            yi = y.bitcast(dt.int32)
            nc.vector.tensor_scalar(yi, ms.bitcast(dt.int32), 1, None,
                                    ALU.logical_shift_right)
            nc.vector.tensor_scalar(yi, yi, -1, 0x5F3759DF, ALU.mult, ALU.add)
            t1 = small.tile([P, 2], dt.float32, tag="t1")
            nc.vector.tensor_mul(t1, y, y)
            nc.vector.tensor_mul(t1, t1, ms)
            nc.vector.tensor_scalar(t1, t1, -8.0, 24.0, ALU.mult, ALU.add)
            nc.vector.tensor_mul(y, y, t1)
            return y

        def rope(px, r, ctab, stab, dst):
            """dst (bf16) = (px*r)*ctab + shuffle(px*r)*stab, all on DVE."""
            a = work.tile([P, D], dt.float32, tag="ra")
            b = work.tile([P, D], dt.float32, tag="rb")
            nc.vector.scalar_tensor_tensor(a, px, r, ctab, ALU.mult, ALU.mult)
            nc.vector.scalar_tensor_tensor(b[:, 0:HD], px[:, HD:D], r,
                                           stab[:, 0:HD], ALU.mult, ALU.mult)
            nc.vector.scalar_tensor_tensor(b[:, HD:D], px[:, 0:HD], r,
                                           stab[:, HD:D], ALU.mult, ALU.mult)
            nc.vector.tensor_add(dst, a, b)

        def proj_matmuls(m):
            quarter = hq[m // 2]
            toff = (m % 2) * P
            cqt = tpool.tile([P, D], dt.float32, tag="cq")
            nc.sync.dma_start(out=cqt, in_=cq[m * P:(m + 1) * P, :])
            sqt = tpool.tile([P, D], dt.float32, tag="sq")
            nc.sync.dma_start(out=sqt, in_=sq[m * P:(m + 1) * P, :])
            ckt = tpool.tile([P, D], dt.float32, tag="ck")
            nc.sync.dma_start(out=ckt, in_=ck[m * P:(m + 1) * P, :])
            skt = tpool.tile([P, D], dt.float32, tag="sk")
            nc.sync.dma_start(out=skt, in_=sk[m * P:(m + 1) * P, :])

            pqk = ph1_ps.tile([P, 2 * D], dt.float32, tag="ph1", name="pqk")
            for t in range(KT):
                nc.tensor.matmul(pqk, lhsT=quarter[:, t, toff:toff + P],
                                 rhs=wqk_sb[t // 5][:, t % 5, :],
                                 start=(t == 0), stop=(t == KT - 1))
            pv = ph1_ps.tile([P, D], dt.float32, tag="ph1", name="pv")
            for t in range(KT):
                nc.tensor.matmul(pv, lhsT=quarter[:, t, toff:toff + P],
                                 rhs=wv_sb[t // 5][:, t % 5, :],
                                 start=(t == 0), stop=(t == KT - 1))
            return pqk, pv, cqt, sqt, ckt, skt

        def norm_rope(m, pqk, pv, cqt, sqt, ckt, skt):
            ssq = small.tile([P, 2], dt.float32, tag="ssq")
            scr = work.tile([P, D], dt.float32, tag="scr")
            nc.scalar.activation(scr, pqk[:, 0:D], ACTF.Square,
                                 accum_out=ssq[:, 0:1])
            scr2 = work.tile([P, D], dt.float32, tag="scr")
            nc.scalar.activation(scr2, pqk[:, D:2 * D], ACTF.Square,
                                 accum_out=ssq[:, 1:2])

            r16 = rsqrt16(ssq)

            qf = work.tile([P, D], dt.bfloat16, tag="qf")
            rope(pqk[:, 0:D], r16[:, 0:1], cqt, sqt, qf)
            kf = work.tile([P, D], dt.bfloat16, tag="kf")
            rope(pqk[:, D:2 * D], r16[:, 1:2], ckt, skt, kf)
            nc.vector.tensor_copy(v_pieces[m], pv)
            return qf, kf

        def transposes(m, qf, kf):
            pi, half = m // 2, (m % 2) * P
            for dh in range(2):
                tp = ph1_ps.tile([P, P], dt.bfloat16, tag="ph1", name="tp")
                nc.tensor.transpose(tp, qf[:, dh * HD:(dh + 1) * HD], ident)
                nc.vector.tensor_copy(qT_pieces[pi][:, dh, half:half + P], tp)
                tp2 = ph1_ps.tile([P, P], dt.bfloat16, tag="ph1", name="tp2")
                nc.tensor.transpose(tp2, kf[:, dh * HD:(dh + 1) * HD], ident)
                nc.vector.tensor_copy(kT_pieces[pi][:, dh, half:half + P], tp2)

        def pair_kks(pp):
            b = 2 * pp
            return b, list(range(max(0, b - 4), b + 2))

        def attn_scores(pp):
            """Produce the masked exp(softcap) probabilities P^T for pair
            pp into its pt tile."""
            b, kks = pair_kks(pp)
            qT = qT_pieces[pp]
            pt = ptp.tile([P, 6, 2 * P], dt.bfloat16, tag="pt", name=f"pt{pp % 2}")
            for j, kk in enumerate(kks):
                st = st_ps.tile([P, 2 * P], dt.float32, tag="st")
                for dh in range(2):
                    nc.tensor.matmul(
                        st,
                        lhsT=kT_pieces[kk // 2][:, dh, (kk % 2) * P:(kk % 2) * P + P],
                        rhs=qT[:, dh, :],
                        start=(dh == 0), stop=(dh == 1))
                th = work.tile([P, 2 * P], dt.float32, tag="th")
                nc.scalar.activation(th, st, ACTF.Tanh, scale=0.02)
                nc.scalar.activation(pt[:, j, :], th, ACTF.Exp, scale=50.0)
                rel = b - kk
                mi = {0: 1, -1: 0, 4: 2, 3: 3}.get(rel)
                if mi is not None:
                    nc.vector.tensor_mul(pt[:, j, :], pt[:, j, :], msk_sb[:, mi, :])
            return pt

        def attn_out(pp, pt):
            """Key-sums, attn@V, o_proj; unnormalized psum -> DRAM."""
            b, kks = pair_kks(pp)
            n_kk = len(kks)
            oTs = [att_ps.tile([P, 2 * P], dt.float32, tag="att", name=f"oT{dh}")
                   for dh in range(2)]
            sums = [att_ps.tile([P, 1], dt.float32, tag="att", name=f"sums{c}")
                    for c in range(2)]
            for j, kk in enumerate(kks):
                for col in range(2):
                    nc.tensor.matmul(sums[col],
                                     lhsT=pt[:, j, col * P:(col + 1) * P],
                                     rhs=ones_col,
                                     start=(j == 0), stop=(j == n_kk - 1))
                for dh in range(2):
                    nc.tensor.matmul(oTs[dh],
                                     lhsT=v_pieces[kk][:, dh * P:(dh + 1) * P],
                                     rhs=pt[:, j, :],
                                     start=(j == 0), stop=(j == n_kk - 1))
            sums_sb = small.tile([P, 2], dt.float32, tag="sums_sb")
            nc.vector.tensor_copy(sums_sb[:, 0:1], sums[0])
            nc.vector.tensor_copy(sums_sb[:, 1:2], sums[1])
            for col in range(2):
                nc.sync.dma_start(out=sums_d[(b + col) * P:(b + col + 1) * P],
                                  in_=sums_sb[:, col:col + 1])
            oT_sb = work.tile([P, 2, 2 * P], dt.bfloat16, tag="oTsb")
            nc.vector.tensor_copy(oT_sb[:, 0, :], oTs[0])
            nc.vector.tensor_copy(oT_sb[:, 1, :], oTs[1])

            for blk in range(2):
                osb = outp.tile([P, HID], dt.bfloat16, tag="osb")
                for nch in range(5):
                    fin = att_ps.tile([P, 512], dt.float32, tag="att", name="fin")
                    for dh in range(2):
                        nc.tensor.matmul(
                            fin,
                            lhsT=oT_sb[:, dh, blk * P:(blk + 1) * P],
                            rhs=wo_sb[:, dh, nch * 512:(nch + 1) * 512],
                            start=(dh == 0), stop=(dh == 1))
                    dst = osb[:, nch * 512:(nch + 1) * 512]
                    if nch % 2 == 0:
                        nc.scalar.copy(dst, fin)
                    else:
                        nc.vector.tensor_copy(dst, fin)
                nc.sync.dma_start(out=out[(b + blk) * P:(b + blk + 1) * P, :],
                                  in_=osb)

        # schedule: attention work for the PREVIOUS pair is emitted
        # between a tile's projection matmuls and its transposes, so the
        # PE has real work to chew on while the norm/rope chain (ACT+DVE)
        # produces the transpose inputs. Remaining const DMAs are
        # emitted just-in-time so the initial burst doesn't starve tile 0.
        pts = {}
        for m in range(NT):
            t = m // 2
            pk = proj_matmuls(m)
            if m == 0:
                for i in range(4):
                    nc.sync.dma_start(out=msk_sb[:, i, :], in_=msk[i])
            if m == 1:
                nc.sync.dma_start(out=wo_sb,
                                  in_=wo.rearrange("(g p) c -> p g c", p=P))
            if m % 2 == 0 and m < NT - 2:
                load_eighth(m // 2 + 1)
            qkf = norm_rope(m, *pk)
            if m % 2 == 0:
                if t >= 1:
                    pts[t - 1] = attn_scores(t - 1)
            else:
                if t >= 1:
                    attn_out(t - 1, pts.pop(t - 1))
            transposes(m, *qkf)
        pts[NPAIR - 1] = attn_scores(NPAIR - 1)
        attn_out(NPAIR - 1, pts.pop(NPAIR - 1))

    return nc


def _host_prep(hidden_states, position_ids, cos_table, sin_table,
               Wq, Wk, Wv, Wo, q_norm_w, k_norm_w):
    f32 = np.float32
    hidden = np.asarray(hidden_states, f32).reshape(S, HID)
    pos = np.asarray(position_ids).reshape(B, S)[0].astype(np.int64)
    cos_g = np.asarray(cos_table, f32)[pos]          # [S, D]
    sin_g = np.asarray(sin_table, f32)[pos]
    qw = 1.0 + np.asarray(q_norm_w, f32)
    kw = 1.0 + np.asarray(k_norm_w, f32)

    sc = f32(SCALE)
    cq = (cos_g * qw * sc).astype(f32)
    sq = np.concatenate([-sin_g[:, :HD] * qw[HD:] * sc,
                         sin_g[:, HD:] * qw[:HD] * sc], axis=1).astype(f32)
    ck = (cos_g * kw).astype(f32)
    sk = np.concatenate([-sin_g[:, :HD] * kw[HD:],
                         sin_g[:, HD:] * kw[:HD]], axis=1).astype(f32)

    hT_t = np.ascontiguousarray(hidden.T).astype(BF16)   # [HID, S]

    jj = np.arange(P)[:, None]
    qi = np.arange(P)[None, :]
    lt = (jj <= qi).astype(f32)
    ut = (jj > qi).astype(f32)
    z = np.zeros((P, P), f32)
    o = np.ones((P, P), f32)
    masks = np.stack([
        np.concatenate([z, lt], axis=1),   # 0: kk == b+1 (rel -1)
        np.concatenate([lt, o], axis=1),   # 1: rel 0
        np.concatenate([ut, z], axis=1),   # 2: rel 4 (left ut, right dead)
        np.concatenate([o, ut], axis=1),   # 3: rel 3 (left full, right ut)
    ]).astype(BF16)

    Wq_ = np.asarray(Wq, f32)
    Wk_ = np.asarray(Wk, f32)
    Wv_ = np.asarray(Wv, f32)
    Wo_ = np.asarray(Wo, f32)

    shared = dict(hT=hT_t, cq=cq, sq=sq, ck=ck, sk=sk, msk=masks)
    in_maps = []
    for h in range(H):
        g = h // (H // KV)
        wq_h = Wq_[h * D:(h + 1) * D, :].T          # [HID, D]
        wk_g = Wk_[g * D:(g + 1) * D, :].T
        in_maps.append(dict(
            shared,
            wqk=np.ascontiguousarray(
                np.concatenate([wq_h, wk_g], axis=1)).astype(BF16),
            wv=np.ascontiguousarray(Wv_[g * D:(g + 1) * D, :].T).astype(BF16),
            wo=np.ascontiguousarray(Wo_[:, h * D:(h + 1) * D].T).astype(BF16),
        ))
    return in_maps


def get_nc():
    if "nc" not in _CACHE:
        _CACHE["nc"] = _build_nc()
    return _CACHE["nc"]


def kernel(hidden_states, position_ids, cos_table, sin_table,
           Wq, Wk, Wv, Wo, q_norm_w, k_norm_w):
    from concourse.bass_utils import run_bass_kernel_spmd

    nc = get_nc()
    in_maps = _host_prep(hidden_states, position_ids, cos_table, sin_table,
                         Wq, Wk, Wv, Wo, q_norm_w, k_norm_w)
    res = run_bass_kernel_spmd(nc, in_maps, list(range(H)))
    acc = np.zeros((S, HID), np.float32)
    for h in range(H):
        r = res.results[h]
        acc += r["out"].astype(np.float32) * (1.0 / r["sums"])[:, None]
    return acc.reshape(B, S, HID)


# revision 32
# speedup vs baseline: 1.3981x; 1.0042x over previous
"""Gemma3 sliding-window attention on 8 trn2 NeuronCores.

Sharding: tensor-parallel over the 8 query heads (1 head per core; each
core recomputes its KV head's k/v projection — no collectives). The host
pre-transposes/pre-tiles inputs into bf16; each core returns its head's
UNNORMALIZED o_proj partial [S, HID] plus per-token softmax sums; the
host applies the division and sums the 8 partials in f32.

Device kernel (identical program on all cores, different weight data):
  phase 1 (per 128-token tile): fused q|k projection (N=512) + v
  projection on PE from a resident hidden^T, RMS-norm via
  ACT-Square+accum and a DVE fast-inverse-sqrt (keeps the whole kernel
  in the `exp_and_others` ACT table set), RoPE with host-folded
  (1+w)*cos/sin*scale tables, PE transposes of q,k into [d, tok].
  phase 2 (per 256-token query pair): S^T = K Q^T on PE so the softmax
  needs no per-block transposes; exp(50*tanh(S^T/50)) with no
  max-subtraction (softcap bounds scores), 0/1 band masks, key-axis
  sums via ones-matmul, attn@V and o_proj on PE, and the final psum is
  DMA'd straight to DRAM (normalization deferred to the host).
"""

import numpy as np
import ml_dtypes

B, S, HID = 1, 2048, 2560
H, KV, D = 8, 4, 256
SCALE = 256 ** -0.5
EPS = 1e-6
P = 128
HD = D // 2          # 128, rotate_half split
NT = S // P          # 16 token tiles
KT = HID // P        # 20 contraction tiles
NPAIR = NT // 2      # 8 query-block pairs
BF16 = ml_dtypes.bfloat16

_CACHE: dict = {}


def _split_multiwait(nc):
    """walrus in this container accepts at most ONE sync wait per
    instruction; hoist extras onto wait-only EventSemaphore instructions
    inserted just before, on the same engine (same program-order
    semantics: waits are >= conditions on monotonic semaphores)."""
    import concourse.mybir as mybir

    n_new = 0
    for fn in nc.m.functions:
        for bb in fn.blocks:
            il = bb.instructions
            out = []
            for ins in il:
                si = ins.sync_info
                if si is not None and si.on_wait and len(si.on_wait) > 1:
                    waits = list(si.on_wait)
                    for w in waits[:-1]:
                        nop = mybir.InstEventSemaphore(
                            name=f"{ins.name}-hw{n_new}", ins=[], outs=[])
                        n_new += 1
                        nop.engine = ins.engine
                        nop.sync_info = mybir.SyncInfo(on_wait=[w], on_update=[])
                        nc.register_instruction(nop, overwrite=True)
                        out.append(nop)
                    ins.sync_info = mybir.SyncInfo(
                        on_wait=[waits[-1]], on_update=list(si.on_update))
                out.append(ins)
            il[:] = out


def _patch_tile_drain():
    """walrus in this container rejects multi-wait instructions; split the
    TileContext exit-drain waits into single wait_ge ops and run a
    whole-module multi-wait split pass at the very end of scheduling."""
    import concourse.mybir as mybir
    import concourse.tile as tile

    if getattr(tile.TileContext, "_drain_patched", False):
        return

    def _patched(self, tick_clock, wait_clock):
        from concourse.tile import ScopedClock

        tmp = mybir.InstNoOp(name="tmp-waits", ins=[], outs=[])
        tmp.engine = mybir.EngineType.SP
        wait_clock.add_sem_waits(tmp, ScopedClock({None: tick_clock.global_clock}))
        by_num = {h.num: h for h in self.sems.allocated().values()}
        for w in (tmp.sync_info.on_wait if tmp.sync_info else []):
            self.nc.sync.wait_ge(by_num[w.id], w.wait_value)
        self.nc.sync.drain()
        self.nc.all_engine_barrier()
        popped = self.nc._tile_sem_poison_stack.pop()
        assert popped is self._sem_poison
        self.nc.clear_and_free_semaphores(list(self.sems.allocated().values()))
        self.nc.all_engine_barrier(sem_only=True)
        _split_multiwait(self.nc)

    tile.TileContext._drain_and_barrier = _patched
    tile.TileContext._drain_patched = True


def _build_nc():
    import concourse.bass as bass
    import concourse.mybir as mybir
    import concourse.tile as tile
    from concourse.masks import make_identity

    _patch_tile_drain()
    dt = mybir.dt
    ALU = mybir.AluOpType
    ACTF = mybir.ActivationFunctionType

    nc = bass.Bass("TRN2", target_bir_lowering=False, debug=False)

    hT = nc.dram_tensor("hT", [HID, S], dt.bfloat16, kind="ExternalInput").ap()
    wqk = nc.dram_tensor("wqk", [HID, 2 * D], dt.bfloat16, kind="ExternalInput").ap()
    wv = nc.dram_tensor("wv", [HID, D], dt.bfloat16, kind="ExternalInput").ap()
    wo = nc.dram_tensor("wo", [D, HID], dt.bfloat16, kind="ExternalInput").ap()
    cq = nc.dram_tensor("cq", [S, D], dt.float32, kind="ExternalInput").ap()
    sq = nc.dram_tensor("sq", [S, D], dt.float32, kind="ExternalInput").ap()
    ck = nc.dram_tensor("ck", [S, D], dt.float32, kind="ExternalInput").ap()
    sk = nc.dram_tensor("sk", [S, D], dt.float32, kind="ExternalInput").ap()
    msk = nc.dram_tensor("msk", [4, P, 2 * P], dt.bfloat16, kind="ExternalInput").ap()
    out = nc.dram_tensor("out", [S, HID], dt.bfloat16, kind="ExternalOutput").ap()
    sums_d = nc.dram_tensor("sums", [S], dt.float32, kind="ExternalOutput").ap()

    from contextlib import ExitStack

    with tile.TileContext(nc) as tc, ExitStack() as ctx:
        consts = ctx.enter_context(tc.tile_pool(name="consts", bufs=1))
        seq = ctx.enter_context(tc.tile_pool(name="seq", bufs=1))
        tpool = ctx.enter_context(tc.tile_pool(name="tabs", bufs=4))
        work = ctx.enter_context(tc.tile_pool(name="work", bufs=3))
        small = ctx.enter_context(tc.tile_pool(name="small", bufs=3))
        ptp = ctx.enter_context(tc.tile_pool(name="ptp", bufs=2))
        outp = ctx.enter_context(tc.tile_pool(name="outp", bufs=3))
        # PSUM budget is 8 banks; every open accumulation group needs its
        # own bank (start= marks the whole 2KB zero-region pending).
        ph1_ps = ctx.enter_context(tc.tile_pool(name="ph1_ps", bufs=2, space="PSUM"))
        st_ps = ctx.enter_context(tc.tile_pool(name="st_ps", bufs=2, space="PSUM"))
        att_ps = ctx.enter_context(tc.tile_pool(name="att_ps", bufs=4, space="PSUM"))

        # ---- constants / resident inputs ----
        # DMA emission is staggered: only what tile 0 needs goes first
        # (quarter 0 of hidden^T + weights); the rest is emitted inside
        # the main loop so the initial burst doesn't starve tile 0.
        hq = [consts.tile([P, KT, S // 8], dt.bfloat16, tag=f"hq{i}",
                          name=f"hq{i}") for i in range(8)]

        def load_eighth(i):
            nc.sync.dma_start(
                out=hq[i],
                in_=hT[:, i * (S // 8):(i + 1) * (S // 8)]
                .rearrange("(t p) n -> p t n", p=P))

        # q|k fused weights, 4 contraction-groups so early matmuls start
        # as soon as the first group lands. DMA order: first hidden
        # eighth, then weight groups in contraction order (what tile 0's
        # matmul sequence consumes first).
        load_eighth(0)
        wqk_sb = [consts.tile([P, 5, 2 * D], dt.bfloat16, tag=f"wqk{g}",
                              name=f"wqk{g}") for g in range(4)]
        wv_sb = [consts.tile([P, 5, D], dt.bfloat16, tag=f"wv{g}",
                             name=f"wv{g}") for g in range(4)]
        for g in range(4):
            nc.sync.dma_start(
                out=wqk_sb[g],
                in_=wqk[g * 5 * P:(g + 1) * 5 * P, :]
                .rearrange("(t p) n -> p t n", p=P))
            nc.sync.dma_start(
                out=wv_sb[g],
                in_=wv[g * 5 * P:(g + 1) * 5 * P, :]
                .rearrange("(t p) n -> p t n", p=P))
        wo_sb = consts.tile([P, 2, HID], dt.bfloat16, tag="wo")
        msk_sb = consts.tile([P, 4, 2 * P], dt.bfloat16, tag="msk")
        ident = consts.tile([P, P], dt.bfloat16, tag="ident")
        make_identity(nc, ident)
        ones_col = consts.tile([P, 1], dt.bfloat16, tag="ones")
        nc.gpsimd.memset(ones_col, 1.0)

        # ---- persistent per-sequence pieces (fine-grained deps) ----
        qT_pieces = [seq.tile([P, 2, 2 * P], dt.bfloat16, tag=f"qT{i}", name=f"qT{i}")
                     for i in range(NPAIR)]
        kT_pieces = [seq.tile([P, 2, 2 * P], dt.bfloat16, tag=f"kT{i}", name=f"kT{i}")
                     for i in range(NPAIR)]
        v_pieces = [seq.tile([P, D], dt.bfloat16, tag=f"v{i}", name=f"v{i}")
                    for i in range(NT)]

        def rsqrt16(ssq):
            """[P, 2] f32 sums-of-squares -> 16/sqrt(x + 256*EPS), via
            fast-inverse-sqrt bit trick + 2 Newton iterations on DVE.
            Two Halley-free Newton steps folded: the second step carries
            the x16 scale. One step leaves ~1.7e-3 rel err (fine next to
            bf16), so only one is used."""
            ms = small.tile([P, 2], dt.float32, tag="ms")
            nc.vector.tensor_scalar(ms, ssq, 256.0 * EPS, None, ALU.add)
            y = small.tile([P, 2], dt.float32, tag="y")
            yi = y.bitcast(dt.int32)
            nc.vector.tensor_scalar(yi, ms.bitcast(dt.int32), 1, None,
                                    ALU.logical_shift_right)
            nc.vector.tensor_scalar(yi, yi, -1, 0x5F3759DF, ALU.mult, ALU.add)
            t1 = small.tile([P, 2], dt.float32, tag="t1")
            nc.vector.tensor_mul(t1, y, y)
            nc.vector.tensor_mul(t1, t1, ms)
            nc.vector.tensor_scalar(t1, t1, -8.0, 24.0, ALU.mult, ALU.add)
            nc.vector.tensor_mul(y, y, t1)
            return y

        def rope(px, r, ctab, stab, dst):
            """dst (bf16) = (px*r)*ctab + shuffle(px*r)*stab, all on DVE."""
            a = work.tile([P, D], dt.float32, tag="ra")
            b = work.tile([P, D], dt.float32, tag="rb")
            nc.vector.scalar_tensor_tensor(a, px, r, ctab, ALU.mult, ALU.mult)
            nc.vector.scalar_tensor_tensor(b[:, 0:HD], px[:, HD:D], r,
                                           stab[:, 0:HD], ALU.mult, ALU.mult)
            nc.vector.scalar_tensor_tensor(b[:, HD:D], px[:, 0:HD], r,
                                           stab[:, HD:D], ALU.mult, ALU.mult)
            nc.vector.tensor_add(dst, a, b)

        def proj_matmuls(m):
            quarter = hq[m // 2]
            toff = (m % 2) * P
            cqt = tpool.tile([P, D], dt.float32, tag="cq")
            nc.sync.dma_start(out=cqt, in_=cq[m * P:(m + 1) * P, :])
            sqt = tpool.tile([P, D], dt.float32, tag="sq")
            nc.sync.dma_start(out=sqt, in_=sq[m * P:(m + 1) * P, :])
            ckt = tpool.tile([P, D], dt.float32, tag="ck")
            nc.sync.dma_start(out=ckt, in_=ck[m * P:(m + 1) * P, :])
            skt = tpool.tile([P, D], dt.float32, tag="sk")
            nc.sync.dma_start(out=skt, in_=sk[m * P:(m + 1) * P, :])

            pqk = ph1_ps.tile([P, 2 * D], dt.float32, tag="ph1", name="pqk")
            for t in range(KT):
                nc.tensor.matmul(pqk, lhsT=quarter[:, t, toff:toff + P],
                                 rhs=wqk_sb[t // 5][:, t % 5, :],
                                 start=(t == 0), stop=(t == KT - 1))
            pv = ph1_ps.tile([P, D], dt.float32, tag="ph1", name="pv")
            for t in range(KT):
                nc.tensor.matmul(pv, lhsT=quarter[:, t, toff:toff + P],
                                 rhs=wv_sb[t // 5][:, t % 5, :],
                                 start=(t == 0), stop=(t == KT - 1))
            return pqk, pv, cqt, sqt, ckt, skt

        def norm_rope(m, pqk, pv, cqt, sqt, ckt, skt):
            ssq = small.tile([P, 2], dt.float32, tag="ssq")
            scr = work.tile([P, D], dt.float32, tag="scr")
            nc.scalar.activation(scr, pqk[:, 0:D], ACTF.Square,
                                 accum_out=ssq[:, 0:1])
            scr2 = work.tile([P, D], dt.float32, tag="scr")
            nc.scalar.activation(scr2, pqk[:, D:2 * D], ACTF.Square,
                                 accum_out=ssq[:, 1:2])

            r16 = rsqrt16(ssq)

            qf = work.tile([P, D], dt.bfloat16, tag="qf")
            rope(pqk[:, 0:D], r16[:, 0:1], cqt, sqt, qf)
            kf = work.tile([P, D], dt.bfloat16, tag="kf")
            rope(pqk[:, D:2 * D], r16[:, 1:2], ckt, skt, kf)
            nc.vector.tensor_copy(v_pieces[m], pv)
            return qf, kf

        def transposes(m, qf, kf):
            pi, half = m // 2, (m % 2) * P
            for dh in range(2):
                tp = ph1_ps.tile([P, P], dt.bfloat16, tag="ph1", name="tp")
                nc.tensor.transpose(tp, qf[:, dh * HD:(dh + 1) * HD], ident)
                nc.vector.tensor_copy(qT_pieces[pi][:, dh, half:half + P], tp)
                tp2 = ph1_ps.tile([P, P], dt.bfloat16, tag="ph1", name="tp2")
                nc.tensor.transpose(tp2, kf[:, dh * HD:(dh + 1) * HD], ident)
                nc.vector.tensor_copy(kT_pieces[pi][:, dh, half:half + P], tp2)

        def pair_kks(pp):
            b = 2 * pp
            return b, list(range(max(0, b - 4), b + 2))

        def attn_scores(pp):
            """Produce the masked exp(softcap) probabilities P^T for pair
            pp into its pt tile."""
            b, kks = pair_kks(pp)
            qT = qT_pieces[pp]
            pt = ptp.tile([P, 6, 2 * P], dt.bfloat16, tag="pt", name=f"pt{pp % 2}")
            for j, kk in enumerate(kks):
                st = st_ps.tile([P, 2 * P], dt.float32, tag="st")
                for dh in range(2):
                    nc.tensor.matmul(
                        st,
                        lhsT=kT_pieces[kk // 2][:, dh, (kk % 2) * P:(kk % 2) * P + P],
                        rhs=qT[:, dh, :],
                        start=(dh == 0), stop=(dh == 1))
                th = work.tile([P, 2 * P], dt.float32, tag="th")
                nc.scalar.activation(th, st, ACTF.Tanh, scale=0.02)
                nc.scalar.activation(pt[:, j, :], th, ACTF.Exp, scale=50.0)
                rel = b - kk
                mi = {0: 1, -1: 0, 4: 2, 3: 3}.get(rel)
                if mi is not None:
                    nc.vector.tensor_mul(pt[:, j, :], pt[:, j, :], msk_sb[:, mi, :])
            return pt

        def attn_out(pp, pt):
            """Key-sums, attn@V, o_proj; unnormalized psum -> DRAM."""
            b, kks = pair_kks(pp)
            n_kk = len(kks)
            oTs = [att_ps.tile([P, 2 * P], dt.float32, tag="att", name=f"oT{dh}")
                   for dh in range(2)]
            sums = [att_ps.tile([P, 1], dt.float32, tag="att", name=f"sums{c}")
                    for c in range(2)]
            for j, kk in enumerate(kks):
                for col in range(2):
                    nc.tensor.matmul(sums[col],
                                     lhsT=pt[:, j, col * P:(col + 1) * P],
                                     rhs=ones_col,
                                     start=(j == 0), stop=(j == n_kk - 1))
                for dh in range(2):
                    nc.tensor.matmul(oTs[dh],
                                     lhsT=v_pieces[kk][:, dh * P:(dh + 1) * P],
                                     rhs=pt[:, j, :],
                                     start=(j == 0), stop=(j == n_kk - 1))
            sums_sb = small.tile([P, 2], dt.float32, tag="sums_sb")
            nc.vector.tensor_copy(sums_sb[:, 0:1], sums[0])
            nc.vector.tensor_copy(sums_sb[:, 1:2], sums[1])
            for col in range(2):
                nc.sync.dma_start(out=sums_d[(b + col) * P:(b + col + 1) * P],
                                  in_=sums_sb[:, col:col + 1])
            oT_sb = work.tile([P, 2, 2 * P], dt.bfloat16, tag="oTsb")
            nc.vector.tensor_copy(oT_sb[:, 0, :], oTs[0])
            nc.vector.tensor_copy(oT_sb[:, 1, :], oTs[1])

            for blk in range(2):
                osb = outp.tile([P, HID], dt.bfloat16, tag="osb")
                for nch in range(5):
                    fin = att_ps.tile([P, 512], dt.float32, tag="att", name="fin")
                    for dh in range(2):
                        nc.tensor.matmul(
                            fin,
                            lhsT=oT_sb[:, dh, blk * P:(blk + 1) * P],
                            rhs=wo_sb[:, dh, nch * 512:(nch + 1) * 512],
                            start=(dh == 0), stop=(dh == 1))
                    dst = osb[:, nch * 512:(nch + 1) * 512]
                    if nch % 2 == 0:
                        nc.scalar.copy(dst, fin)
                    else:
                        nc.vector.tensor_copy(dst, fin)
                nc.sync.dma_start(out=out[(b + blk) * P:(b + blk + 1) * P, :],
                                  in_=osb)

        # schedule: attention work for the PREVIOUS pair is emitted
        # between a tile's projection matmuls and its transposes, so the
        # PE has real work to chew on while the norm/rope chain (ACT+DVE)
        # produces the transpose inputs. Remaining const DMAs are
        # emitted just-in-time so the initial burst doesn't starve tile 0.
        pts = {}
        for m in range(NT):
            t = m // 2
            pk = proj_matmuls(m)
            if m == 0:
                for i in range(4):
                    nc.sync.dma_start(out=msk_sb[:, i, :], in_=msk[i])
            if m == 1:
                nc.sync.dma_start(out=wo_sb,
                                  in_=wo.rearrange("(g p) c -> p g c", p=P))
            if m % 2 == 0 and m < NT - 2:
                load_eighth(m // 2 + 1)
            qkf = norm_rope(m, *pk)
            if m % 2 == 0:
                if t >= 1:
                    pts[t - 1] = attn_scores(t - 1)
            else:
                if t >= 1:
                    attn_out(t - 1, pts.pop(t - 1))
            transposes(m, *qkf)
        pts[NPAIR - 1] = attn_scores(NPAIR - 1)
        attn_out(NPAIR - 1, pts.pop(NPAIR - 1))

    return nc


def _host_prep(hidden_states, position_ids, cos_table, sin_table,
               Wq, Wk, Wv, Wo, q_norm_w, k_norm_w):
    f32 = np.float32
    hidden = np.asarray(hidden_states, f32).reshape(S, HID)
    pos = np.asarray(position_ids).reshape(B, S)[0].astype(np.int64)
    cos_g = np.asarray(cos_table, f32)[pos]          # [S, D]
    sin_g = np.asarray(sin_table, f32)[pos]
    qw = 1.0 + np.asarray(q_norm_w, f32)
    kw = 1.0 + np.asarray(k_norm_w, f32)

    sc = f32(SCALE)
    cq = (cos_g * qw * sc).astype(f32)
    sq = np.concatenate([-sin_g[:, :HD] * qw[HD:] * sc,
                         sin_g[:, HD:] * qw[:HD] * sc], axis=1).astype(f32)
    ck = (cos_g * kw).astype(f32)
    sk = np.concatenate([-sin_g[:, :HD] * kw[HD:],
                         sin_g[:, HD:] * kw[:HD]], axis=1).astype(f32)

    hT_t = np.ascontiguousarray(hidden.T).astype(BF16)   # [HID, S]

    jj = np.arange(P)[:, None]
    qi = np.arange(P)[None, :]
    lt = (jj <= qi).astype(f32)
    ut = (jj > qi).astype(f32)
    z = np.zeros((P, P), f32)
    o = np.ones((P, P), f32)
    masks = np.stack([
        np.concatenate([z, lt], axis=1),   # 0: kk == b+1 (rel -1)
        np.concatenate([lt, o], axis=1),   # 1: rel 0
        np.concatenate([ut, z], axis=1),   # 2: rel 4 (left ut, right dead)
        np.concatenate([o, ut], axis=1),   # 3: rel 3 (left full, right ut)
    ]).astype(BF16)

    Wq_ = np.asarray(Wq, f32)
    Wk_ = np.asarray(Wk, f32)
    Wv_ = np.asarray(Wv, f32)
    Wo_ = np.asarray(Wo, f32)

    shared = dict(hT=hT_t, cq=cq, sq=sq, ck=ck, sk=sk, msk=masks)
    in_maps = []
    for h in range(H):
        g = h // (H // KV)
        wq_h = Wq_[h * D:(h + 1) * D, :].T          # [HID, D]
        wk_g = Wk_[g * D:(g + 1) * D, :].T
        in_maps.append(dict(
            shared,
            wqk=np.ascontiguousarray(
                np.concatenate([wq_h, wk_g], axis=1)).astype(BF16),
            wv=np.ascontiguousarray(Wv_[g * D:(g + 1) * D, :].T).astype(BF16),
            wo=np.ascontiguousarray(Wo_[:, h * D:(h + 1) * D].T).astype(BF16),
        ))
    return in_maps


def get_nc():
    if "nc" not in _CACHE:
        _CACHE["nc"] = _build_nc()
    return _CACHE["nc"]


def kernel(hidden_states, position_ids, cos_table, sin_table,
           Wq, Wk, Wv, Wo, q_norm_w, k_norm_w):
    from concourse.bass_utils import run_bass_kernel_spmd

    nc = get_nc()
    in_maps = _host_prep(hidden_states, position_ids, cos_table, sin_table,
                         Wq, Wk, Wv, Wo, q_norm_w, k_norm_w)
    res = run_bass_kernel_spmd(nc, in_maps, list(range(H)))
    acc = np.zeros((S, HID), np.float32)
    for h in range(H):
        r = res.results[h]
        acc += r["out"].astype(np.float32) * (1.0 / r["sums"])[:, None]
    return acc.reshape(B, S, HID)
